# revision 62
# baseline (speedup 1.0000x reference)
"""Trainium2 Bass kernel for nn_AttentionLayer (B=4, S=2048, H=12, D=64).

Sharding: 8 cores = 4 batches x 2 head-groups (6 heads each).
Per core: QKV projections for its 384 W-columns, then per-(head) attention
with a UniLM prefix "staircase" mask.  Fully-masked [128k x 512q] tiles are
skipped at program-build time (union over the 4 batches); partially-masked
tiles get a multiplicative 0/1 mask after exp, and only the visible
q-suffix is computed.

v3 structure (why it is shaped this way):
- The PE is output-column-bound (1 psum col/cycle @2.4GHz) regardless of
  dtype; fp8 DoubleRow's win is CONTRACTION DEPTH (256 rows/pass), not
  speed.  So: scores stay bf16 (64-deep, nothing to gain), while the
  q/k/v projections (768-deep) run as fp8e4 DoubleRow over feature-chunk
  pairs - half the passes.  ctx keeps the baseline fp8 DoubleRow k-chunk
  pairing.  Quanta feeding the error-sensitive qb=0 block (peaked
  queries that set the global max) stay bf16.
- exp is the other wall (ACT ~1 elem/cycle/partition @1.2GHz).  A
  Schraudolph DVE exp path exists (bf16 bits = int16 round of an affine
  of the score via a bitcast view; numerically validated, rel ~4.2e-3)
  but is DISABLED: in every placement tried (global 20%, tail-only 15%)
  the DVE FIFO latency it adds to the exp->mask->ctx chains cost more
  wall clock than the ACT relief bought (the kernel is chain-paced, not
  engine-throughput-paced, at the margin).
- Projection matmuls are emitted as PE filler inside the attention loop
  (keeps PE duty high so the HAM clock gate never drops the PE to
  1.2GHz); ctx runs lag-1 behind scores; softmax normalization runs off
  the PE (fast reciprocal + GPSIMD partition broadcast).
"""

import sys

if "/opt/trn_rl_repo" not in sys.path:
    sys.path.insert(0, "/opt/trn_rl_repo")

from contextlib import ExitStack

import ml_dtypes
import numpy as np

import concourse.bass as bass
import concourse.mybir as mybir
import concourse.tile as tile
from concourse import bacc, library_config
from concourse.bass_utils import run_bass_kernel_spmd

B, S, W, H, D = 4, 2048, 768, 12, 64
NCORES = 8
HPC = 6  # heads per core
QB = 512  # q block (free dim of a scores tile)
KC = 128  # k chunk (partition dim of a scores tile)
NQB = S // QB
NKC = S // KC
MC = 3  # 128-row chunks of the 384 per-core W-columns
FKC = W // 128  # feature chunks (contraction for projections)
HD = HPC * D  # 384
MB = FKC * 128  # one mc block of wq/wk columns
VE = 80  # per-head pitch in v_aug (64 d + 1 ones + pad to 16B for DoubleRow)
VW = HPC * VE  # v_aug row width per k-chunk
VW16 = HPC * (D + 1)
ACT_GROUP = 2  # k-chunks per ACT instruction = one fp8 DoubleRow ctx pair

F32 = mybir.dt.float32
BF16 = mybir.dt.bfloat16
FP8 = mybir.dt.float8e4
I16 = mybir.dt.int16
DR = mybir.MatmulPerfMode.DoubleRow

# Schraudolph exp for the DVE path: bf16 bits = round(x * 128*log2(e)/8 + b)
# (1/sqrt(D) folded into the scale; +0.5 emulates round on the truncating
# float->int16 convert).
SCH_A = 128.0 * 1.4426950408889634 / 8.0
SCH_B = 128.0 * (127.0 - 0.0450) + 0.5
DVE_EXP_FRAC = 0.0  # Schraudolph exp offload: hurt in every placement tried
DVE_HP_MIN = 2
XBQ = 2  # bf16 xt S-slices kept (nb0/nb1 only feed bf16 quanta)

TRACE = False  # set by test.py to profile
LAST_RESULTS = None  # BassKernelResults of the last run (for test.py)


def _ensure_ntff_hook():
    """This image's antenv lacks axon_hooks; register the ctypes NTFF
    profile hook from trn_agent_boot ourselves so trace=True works."""
    import types

    if "antenv.axon_hooks" in sys.modules:
        return
    try:
        from trn_agent_boot.trn_boot import _ntff_profile_via_ctypes

        hook = _ntff_profile_via_ctypes("/opt/axon/libaxon_pjrt.so")
    except Exception:
        hook = None
    mod = types.ModuleType("antenv.axon_hooks")
    mod._hook = hook
    mod.set_axon_ntff_profile_hook = lambda h: setattr(mod, "_hook", h)
    mod.get_axon_ntff_profile_hook = lambda: mod._hook
    sys.modules["antenv.axon_hooks"] = mod
    # artifact upload needs egress this sandbox doesn't have
    import concourse.bass_utils as _bu

    _bu.upload_artifacts = lambda d: "local://" + str(d)


def _classify(seg):
    """Union-over-batches tile classification from segment_ids."""
    cs = np.cumsum(np.asarray(seg, np.int64), axis=1)
    vis_lists = [[] for _ in range(NQB)]
    bnd_index = {}
    q0map = {}
    q1map = {}
    for qb in range(NQB):
        for kc in range(NKC):
            any_computed = False
            all_full_vis = True
            q0u, q1u = QB, 0
            for b in range(B):
                c = cs[b]
                full_mask = c[kc * KC] > c[qb * QB + QB - 1]
                full_vis = c[kc * KC + KC - 1] <= c[qb * QB]
                if not full_mask:
                    any_computed = True
                if not full_vis:
                    all_full_vis = False
                qcs = c[qb * QB : (qb + 1) * QB]
                anyv = np.nonzero(qcs >= c[kc * KC])[0]
                fullv = np.nonzero(qcs >= c[kc * KC + KC - 1])[0]
                q0u = min(q0u, int(anyv[0]) if len(anyv) else QB)
                q1u = max(q1u, int(fullv[0]) if len(fullv) else QB)
            if any_computed:
                vis_lists[qb].append(kc)
                if not all_full_vis:
                    bnd_index[(kc, qb)] = True
                    q0map[(kc, qb)] = (q0u // 16) * 16
                    q1map[(kc, qb)] = min(QB, ((q1u + 15) // 16) * 16)
                else:
                    q0map[(kc, qb)] = 0
                    q1map[(kc, qb)] = 0
    return cs, vis_lists, bnd_index, (q0map, q1map)


def _plan(vis_lists, bnd_index):
    """Groups, mask tile indices, and the DVE-exp group set."""
    groups = {}
    for hp in range(HPC // 2):
        for qb in range(NQB):
            vis = vis_lists[qb]
            groups[(hp, qb)] = [
                vis[i : i + ACT_GROUP] for i in range(0, len(vis), ACT_GROUP)
            ]
    mi8, mi16 = {}, {}
    for (kc, qb) in bnd_index:
        if qb == 0:
            mi16[(kc, qb)] = len(mi16)
        else:
            mi8[(kc, qb)] = len(mi8)
    total_area = 0
    cand = []
    for hp in range(HPC // 2):
        for qb in range(1, NQB):
            for gi, g in enumerate(groups[(hp, qb)]):
                a = len(g) * QB * KC
                total_area += a
                if hp >= DVE_HP_MIN and all(
                    (kc, qb) not in bnd_index for kc in g
                ):
                    cand.append((hp, qb, gi, a))
    dve_groups = set()
    target = DVE_EXP_FRAC * total_area
    acc = 0
    for hp, qb, gi, a in sorted(cand, key=lambda t: (t[2], t[0], t[1])):
        if acc >= target:
            break
        dve_groups.add((hp, qb, gi))
        acc += a
    return groups, mi8, mi16, dve_groups


def _build_program(vis_lists, bnd_index, qmaps):
    nc = bacc.Bacc()
    q0map, q1map = qmaps
    groups, mi8, mi16, dve_groups = _plan(vis_lists, bnd_index)
    N0 = len(vis_lists[0])
    assert max(vis_lists[0]) < 8, "qb0 visible chunks must sit in k nb0/nb1"
    n_bnd8 = max(len(mi8), 1)
    n_bnd16 = max(len(mi16), 1)
    vb_chunks = set(range(N0))
    for (hp, qb, gi) in dve_groups:
        vb_chunks.update(groups[(hp, qb)][gi])

    XBW = XBQ * QB  # bf16 xt pitch per feature chunk
    xT_d = nc.declare_dram_parameter("xT", [128, FKC * XBW], BF16, isOutput=False)
    xT8_d = nc.declare_dram_parameter("xT8", [128, FKC * S], FP8, isOutput=False)
    wq16_d = nc.declare_dram_parameter("wq16", [128, MC * MB], BF16, isOutput=False)
    wk16_d = nc.declare_dram_parameter("wk16", [128, MC * MB], BF16, isOutput=False)
    wq8_d = nc.declare_dram_parameter("wq8", [128, MC * MB], FP8, isOutput=False)
    wk8_d = nc.declare_dram_parameter("wk8", [128, MC * MB], FP8, isOutput=False)
    wv16_d = nc.declare_dram_parameter("wv16", [128, FKC * HD], BF16, isOutput=False)
    wv8_d = nc.declare_dram_parameter("wv8", [128, FKC * HD], FP8, isOutput=False)
    bqk_d = nc.declare_dram_parameter("bqk", [128, 2 * MC], F32, isOutput=False)
    bvb_d = nc.declare_dram_parameter("bvb", [128, HD], F32, isOutput=False)
    csb_d = nc.declare_dram_parameter("cs_bcast", [128, S], F32, isOutput=False)
    csp_d = nc.declare_dram_parameter("cs_part", [128, NKC], F32, isOutput=False)
    out_d = nc.declare_dram_parameter("ctxT", [MC * 128, S], F32, isOutput=True)

    with ExitStack() as ctx:
        tc = ctx.enter_context(tile.TileContext(nc))
        persist = ctx.enter_context(tc.tile_pool(name="persist", bufs=1))

        qt = persist.tile([128, MC * S], BF16)
        kt = persist.tile([128, MC * S], BF16)
        v = persist.tile([128, NKC * VW], FP8)
        vb16 = persist.tile([128, NKC * VW16], BF16)
        ctxt = persist.tile([128, MC * S], F32)
        msk = persist.tile([128, n_bnd8 * QB], FP8)
        mskb = persist.tile([128, n_bnd16 * QB], BF16)
        cs_b = persist.tile([128, S], F32)
        cs_p = persist.tile([128, NKC], F32)
        bqk_sb = persist.tile([128, 2 * MC], F32)
        bv_sb = persist.tile([128, HD], F32)
        warmsrc = persist.tile([128, 640], BF16)
        nc.vector.memset(warmsrc, 0.0)
        nc.gpsimd.load_library(library_config.attn)  # partition_broadcast ucode

        with (
            tc.tile_pool(name="ld", bufs=1) as ld,
            tc.tile_pool(name="pps", bufs=2, space="PSUM") as pps,
            tc.tile_pool(name="scps", bufs=2, space="PSUM") as scps,
            tc.tile_pool(name="ctxps", bufs=2, space="PSUM") as ctxps,
            tc.tile_pool(name="expp", bufs=6) as expp,
            tc.tile_pool(name="lpool", bufs=4) as lpool,
        ):
            xt = ld.tile([128, FKC * XBW], BF16)
            xt8 = ld.tile([128, FKC * S], FP8)
            wq16_sb = ld.tile([128, MC * MB], BF16)
            wk16_sb = ld.tile([128, MC * MB], BF16)
            wq8_sb = ld.tile([128, MC * MB], FP8)
            wk8_sb = ld.tile([128, MC * MB], FP8)
            wv16_sb = ld.tile([128, FKC * HD], BF16)
            wv8_sb = ld.tile([128, FKC * HD], FP8)
            # load order = first-use order: mc0 bf16 weights + x feed the
            # prelude quanta; wv16 feeds the qb0 v chunks (slots 1-2); cs
            # feeds the first boundary masks; fp8 copies are needed from
            # slot ~3 on; mc1/2 bf16 weights not until hp=1.
            # Sync carries the critical prologue chain (its queue is free at
            # t=0; the Scalar queue starts with ~2.6us of engine/act-table
            # loads).  The fp8 copies ride the Scalar queue (not needed until
            # slot ~3), cs_b rides GPSIMD behind load_library.
            # xt stays per-chunk: the prelude quantum matmuls pipeline with
            # the chunk arrivals (a single merged xt DMA makes the first
            # matmul wait the whole 1.57MB transfer: first exp +12us).  xt8
            # is merged below - not latency-critical, and it frees 5 Sync
            # issue slots.
            nc.sync.dma_start(out=wq16_sb[:, 0:MB], in_=wq16_d[:, 0:MB])
            for kc in range(3):
                nc.sync.dma_start(
                    out=xt[:, kc * XBW : (kc + 1) * XBW],
                    in_=xT_d[:, kc * XBW : (kc + 1) * XBW],
                )
            nc.sync.dma_start(out=bqk_sb, in_=bqk_d[:])
            for kc in range(3, FKC):
                nc.sync.dma_start(
                    out=xt[:, kc * XBW : (kc + 1) * XBW],
                    in_=xT_d[:, kc * XBW : (kc + 1) * XBW],
                )
            nc.sync.dma_start(out=wk16_sb[:, 0:MB], in_=wk16_d[:, 0:MB])
            nc.sync.dma_start(out=wv16_sb, in_=wv16_d[:])
            nc.gpsimd.dma_start(out=cs_b, in_=csb_d[:])
            nc.sync.dma_start(out=cs_p, in_=csp_d[:])
            nc.sync.dma_start(out=bv_sb, in_=bvb_d[:])
            nc.sync.dma_start(out=wq8_sb, in_=wq8_d[:])
            nc.sync.dma_start(out=wk8_sb, in_=wk8_d[:])
            nc.sync.dma_start(out=xt8, in_=xT8_d[:])
            nc.sync.dma_start(out=wv8_sb, in_=wv8_d[:])
            nc.sync.dma_start(out=wq16_sb[:, MB:], in_=wq16_d[:, MB:])
            nc.sync.dma_start(out=wk16_sb[:, MB:], in_=wk16_d[:, MB:])

            xt84 = xt8.rearrange("p (k s) -> p k s", k=FKC)
            wq84 = wq8_sb.rearrange("p (m k c) -> p m k c", m=MC, k=FKC)
            wk84 = wk8_sb.rearrange("p (m k c) -> p m k c", m=MC, k=FKC)
            wv84 = wv8_sb.rearrange("p (k c) -> p k c", k=FKC)

            # masks are built lazily (first use) so the DVE isn't tied up
            # during the prologue while the first qk drains are demanded
            built_masks = set()

            def mask_jit(kc, qb):
                if (kc, qb) in built_masks:
                    return
                built_masks.add((kc, qb))
                if qb == 0:
                    bi, dstm = mi16[(kc, qb)], mskb
                else:
                    bi, dstm = mi8[(kc, qb)], msk
                nc.vector.tensor_scalar(
                    out=dstm[:, bi * QB : (bi + 1) * QB],
                    in0=cs_b[:, qb * QB : (qb + 1) * QB],
                    scalar1=cs_p[:, kc : kc + 1],
                    scalar2=None,
                    op0=mybir.AluOpType.is_ge,
                )

            v4 = v.rearrange("p (s h e) -> p s h e", h=HPC, e=VE)
            nc.vector.memset(v4[:, :, :, D : D + 1], 1.0)
            vb4 = vb16.rearrange("p (s h e) -> p s h e", h=HPC, e=D + 1)
            nc.vector.memset(vb4[:, :, :, D : D + 1], 1.0)
            # pre-touch the fp8 esb ring: stale regions below a split exp's
            # q0 are zeroed by the boundary mask, and 0*NaN would poison ctx
            for i in range(6):
                t8 = expp.tile([128, ACT_GROUP * QB], FP8, tag="esb8", name="z8")
                nc.gpsimd.memset(t8, 0.0)

            # --- projection quanta ---------------------------------------
            # bf16 quanta are 6 matmuls (~1.3us) - the largest PE lump.  When
            # popped as leisure filler they are emitted as two 3-matmul
            # halves across consecutive pops, so the PE is never occupied for
            # a full quantum right when a scores psum buffer releases (that
            # jitter is what opens the ~0.5us/slot gaps in the exp stream).
            half_pending = {}  # fq -> psum tile awaiting kc3-5 + drain

            def _qk_bf16_mms(ps, pi, mc, nb, lo, hi):
                w_sb = wq16_sb if pi == 0 else wk16_sb
                for kc in range(lo, hi):
                    nc.tensor.matmul(
                        ps,
                        lhsT=w_sb[:, mc * MB + kc * 128 : mc * MB + kc * 128 + 128],
                        rhs=xt[:, kc * XBW + nb * QB : kc * XBW + (nb + 1) * QB],
                        start=(kc == 0),
                        stop=(kc == FKC - 1),
                    )

            def _qk_drain(ps, pi, mc, nb, drain_on_act):
                out_sb = qt if pi == 0 else kt
                if drain_on_act:
                    nc.scalar.activation(
                        out=out_sb[:, mc * S + nb * QB : mc * S + (nb + 1) * QB],
                        in_=ps,
                        func=mybir.ActivationFunctionType.Identity,
                        bias=bqk_sb[:, pi * MC + mc : pi * MC + mc + 1],
                        scale=1.0,
                    )
                else:
                    nc.vector.tensor_scalar_add(
                        out_sb[:, mc * S + nb * QB : mc * S + (nb + 1) * QB],
                        ps,
                        bqk_sb[:, pi * MC + mc : pi * MC + mc + 1],
                    )

            def finish_half(fq, drain_on_act=True):
                ps = half_pending.pop(fq)
                _qk_bf16_mms(ps, fq[1], fq[2], fq[3], FKC // 2, FKC)
                _qk_drain(ps, fq[1], fq[2], fq[3], drain_on_act)

            def qk_quantum(pi, mc, nb, drain_on_act=False, first_half_only=False):
                kind_bf16 = (pi == 0 and nb == 0) or (pi == 1 and nb <= 1)
                ps = pps.tile([128, QB], F32, tag="proj", name="psqk")
                if kind_bf16:
                    if first_half_only:
                        _qk_bf16_mms(ps, pi, mc, nb, 0, FKC // 2)
                        half_pending[("qk", pi, mc, nb)] = ps
                        return
                    _qk_bf16_mms(ps, pi, mc, nb, 0, FKC)
                else:
                    w4 = wq84 if pi == 0 else wk84
                    for j in range(FKC // 2):
                        nc.tensor.matmul(
                            ps,
                            lhsT=w4[:, mc, 2 * j : 2 * j + 2, :],
                            rhs=xt84[:, 2 * j : 2 * j + 2, nb * QB : (nb + 1) * QB],
                            start=(j == 0),
                            stop=(j == FKC // 2 - 1),
                            perf_mode=DR,
                        )
                _qk_drain(ps, pi, mc, nb, drain_on_act)

            def v_quantum(sc):
                ps = pps.tile([128, HD], F32, tag="proj", name="psv")
                if sc < N0:
                    for kc in range(FKC):
                        nc.tensor.matmul(
                            ps,
                            lhsT=xt[:, kc * XBW + sc * KC : kc * XBW + sc * KC + KC],
                            rhs=wv16_sb[:, kc * HD : (kc + 1) * HD],
                            start=(kc == 0),
                            stop=(kc == FKC - 1),
                        )
                else:
                    for j in range(FKC // 2):
                        nc.tensor.matmul(
                            ps,
                            lhsT=xt84[:, 2 * j : 2 * j + 2, sc * KC : sc * KC + KC],
                            rhs=wv84[:, 2 * j : 2 * j + 2, :],
                            start=(j == 0),
                            stop=(j == FKC // 2 - 1),
                            perf_mode=DR,
                        )
                nc.vector.tensor_add(
                    v4[:, sc, :, 0:D],
                    ps.rearrange("p (h e) -> p h e", e=D),
                    bv_sb.rearrange("p (h e) -> p h e", e=D),
                )
                if sc in vb_chunks:
                    nc.vector.tensor_add(
                        vb4[:, sc, :, 0:D],
                        ps.rearrange("p (h e) -> p h e", e=D),
                        bv_sb.rearrange("p (h e) -> p h e", e=D),
                    )

            emitted = set()

            def flush_halves(drain_on_act=True):
                # a pending half holds a pps ring buffer; its completing
                # matmuls+drain must precede any further pps allocation in
                # the in-order PE FIFO or a later allocation can deadlock
                for f in list(half_pending):
                    finish_half(f, drain_on_act)
                    emitted.add(f)

            def emit_quantum(fq, drain_on_act=False, first_half=False):
                if fq in emitted or fq in half_pending:
                    return
                flush_halves()
                kind_bf16 = fq[0] == "qk" and (
                    (fq[1] == 0 and fq[3] == 0) or (fq[1] == 1 and fq[3] <= 1)
                )
                if fq[0] == "v":
                    emitted.add(fq)
                    v_quantum(fq[1])
                    return
                if first_half and kind_bf16:
                    qk_quantum(fq[1], fq[2], fq[3], drain_on_act, first_half_only=True)
                    return
                emitted.add(fq)
                qk_quantum(fq[1], fq[2], fq[3], drain_on_act)

            # filler order: mc0 fp8 quanta, then the HEAVY bf16 quanta of
            # mc1/mc2 spread early (so the hp transitions never demand a
            # burst of 1.3us quanta at once), then v tail, then fp8 mc1/2.
            filler = []
            for nb in range(1, NQB):
                filler.append(("qk", 0, 0, nb))
            for nb in range(2, NQB):
                filler.append(("qk", 1, 0, nb))
            for sc in range(N0, N0 + 4):
                filler.append(("v", sc))
            for mc in (1, 2):
                filler.append(("qk", 0, mc, 0))
                filler.append(("qk", 1, mc, 0))
                filler.append(("qk", 1, mc, 1))
            for sc in range(N0 + 4, NKC):
                filler.append(("v", sc))
            for mc in (1, 2):
                for pi in range(2):
                    for nb in range(1 if pi == 0 else 2, NQB):
                        filler.append(("qk", pi, mc, nb))

            demand_q = []

            def require(fq):
                if fq in emitted:
                    return
                if fq in half_pending:
                    finish_half(fq, drain_on_act=False)  # demanded: DVE drain
                    emitted.add(fq)
                    return
                if fq in filler:
                    filler.remove(fq)
                if fq in demand_q:
                    demand_q.remove(fq)
                emit_quantum(fq)

            def require_soon(fq):
                if fq in emitted or fq in half_pending or fq in demand_q:
                    return
                if fq in filler:
                    filler.remove(fq)
                demand_q.append(fq)

            def warm_dummy():
                ps = pps.tile([128, QB], F32, tag="proj", name="warm")
                nc.tensor.matmul(
                    ps,
                    lhsT=warmsrc[:, 0:128],
                    rhs=warmsrc[:, 128 : 128 + QB],
                    start=True,
                    stop=True,
                )

            def leisure_pop():
                if half_pending:
                    flush_halves()
                elif demand_q:
                    emit_quantum(demand_q.pop(0))
                elif filler:
                    # NB: half-quantum smoothing (first_half=True) tested
                    # 3.3us WORSE than whole-quantum pops; keep whole.
                    emit_quantum(filler.pop(0), drain_on_act=True)
                else:
                    for _ in range(4):
                        warm_dummy()

            def scores_needs(hp, qb, g):
                needs = [("qk", 0, hp, qb)]
                nb_hi = (g[-1] * KC + KC - 1) // QB
                for nb in range(nb_hi + 1):
                    needs.append(("qk", 1, hp, nb))
                return needs

            def group_needs(hp, qb, g):
                return scores_needs(hp, qb, g) + [("v", kc) for kc in g]

            emit_quantum(("qk", 0, 0, 0))
            emit_quantum(("qk", 1, 0, 0))
            for sc in range(N0):
                filler.insert(sc, ("v", sc))

            # --- attention -----------------------------------------------
            def emit_scores_group(hp, qb, g, gi):
                if qb == 0:
                    path = "bf16"
                elif (hp, qb, gi) in dve_groups:
                    path = "dve"
                else:
                    path = "fp8"
                q0s = [q0map[(kc, qb)] for kc in g]
                gq0 = min(q0s)
                # per-chunk exp ranges only on the fp8 path (its esb ring is
                # pre-zeroed; stale [gq0,q0c) is masked to 0, never NaN)
                exp_split = (max(q0s) - gq0) >= 224 and path == "fp8"
                eq0 = [q0 if (exp_split or q0 == gq0) else gq0 for q0 in q0s]

                for kc in g:
                    if (kc, qb) in bnd_index:
                        mask_jit(kc, qb)
                mcq = hp
                sps = {}
                esb = {}
                etag = {"bf16": "esb16b", "fp8": "esb8", "dve": "esb16"}[path]
                edt = FP8 if path == "fp8" else BF16
                for par in range(2):
                    sps[par] = scps.tile(
                        [128, ACT_GROUP * QB], F32, tag="sps", name=f"sps{par}"
                    )
                    esb[par] = expp.tile(
                        [128, ACT_GROUP * QB], edt, tag=etag, name=f"esb{par}"
                    )
                # par-outer: par0's scores finish 1 matmul earlier, so its
                # exp (the slot's pacing chain) starts sooner; each par's
                # masks follow its own exp so ctx(par0) is ready while
                # exp(par1) still runs.
                scale = 1.0 / float(np.sqrt(np.float32(D)))
                for par in range(2):
                    po = par * 64
                    for j, kc in enumerate(g):
                        nc.tensor.matmul(
                            sps[par][:, j * QB + eq0[j] : (j + 1) * QB],
                            lhsT=kt[
                                po : po + 64, mcq * S + kc * KC : mcq * S + kc * KC + KC
                            ],
                            rhs=qt[
                                po : po + 64,
                                mcq * S + qb * QB + eq0[j] : mcq * S + (qb + 1) * QB,
                            ],
                            start=True,
                            stop=True,
                        )
                for par in range(2):
                    if path == "dve":
                        nc.vector.tensor_scalar(
                            out=esb[par][:, 0 : len(g) * QB].bitcast(I16),
                            in0=sps[par][:, 0 : len(g) * QB],
                            scalar1=SCH_A,
                            scalar2=SCH_B,
                            op0=mybir.AluOpType.mult,
                            op1=mybir.AluOpType.add,
                        )
                    elif exp_split or len(g) == 1:
                        for j in range(len(g)):
                            nc.scalar.activation(
                                out=esb[par][:, j * QB + eq0[j] : (j + 1) * QB],
                                in_=sps[par][:, j * QB + eq0[j] : (j + 1) * QB],
                                func=mybir.ActivationFunctionType.Exp,
                                scale=scale,
                            )
                    else:
                        src = sps[par].rearrange("p (j q) -> p j q", j=2)[:, :, gq0:]
                        dst = esb[par].rearrange("p (j q) -> p j q", j=2)[:, :, gq0:]
                        nc.scalar.activation(
                            out=dst,
                            in_=src,
                            func=mybir.ActivationFunctionType.Exp,
                            scale=scale,
                        )
                    for j, kc in enumerate(g):
                        if (kc, qb) not in bnd_index:
                            continue
                        q1 = q1map[(kc, qb)]
                        if qb == 0:
                            bi, srcm = mi16[(kc, qb)], mskb
                        else:
                            bi, srcm = mi8[(kc, qb)], msk
                        nc.vector.tensor_mul(
                            esb[par][:, j * QB + gq0 : j * QB + q1],
                            esb[par][:, j * QB + gq0 : j * QB + q1],
                            srcm[:, bi * QB + gq0 : bi * QB + q1],
                        )
                return esb, gq0, path

            def emit_ctx_group(hp, qb, g, gq0, path, esb, cps, unit, n_units):
                if path == "fp8":
                    for par in range(2):
                        h = 2 * hp + par
                        if len(g) == 2:
                            nc.tensor.matmul(
                                cps[par][:, gq0:],
                                lhsT=v4[:, g[0] : g[0] + 2, h, 0 : D + 1],
                                rhs=esb[par].rearrange("p (j q) -> p j q", j=2)[
                                    :, :, gq0:
                                ],
                                start=(unit == 0),
                                stop=(unit == n_units - 1),
                                perf_mode=DR,
                            )
                        else:
                            nc.tensor.matmul(
                                cps[par][:, gq0:],
                                lhsT=v4[:, g[0], h, 0 : D + 1],
                                rhs=esb[par][:, gq0:QB],
                                start=(unit == 0),
                                stop=(unit == n_units - 1),
                            )
                    return unit + 1
                for j, kc in enumerate(g):
                    for par in range(2):
                        h = 2 * hp + par
                        nc.tensor.matmul(
                            cps[par][:, gq0:],
                            lhsT=vb16[
                                :, kc * VW16 + h * (D + 1) : kc * VW16 + (h + 1) * (D + 1)
                            ],
                            rhs=esb[par][:, j * QB + gq0 : (j + 1) * QB],
                            start=(unit + j == 0),
                            stop=(unit + j == n_units - 1),
                        )
                return unit + len(g)

            def n_units_for(hp, qb):
                n = 0
                for gi, g in enumerate(groups[(hp, qb)]):
                    if qb == 0 or (hp, qb, gi) in dve_groups:
                        n += len(g)
                    else:
                        n += 1
                return n

            all_slots = []
            for hp in range(HPC // 2):
                for qb in range(NQB):
                    for gi, g in enumerate(groups[(hp, qb)]):
                        all_slots.append((hp, qb, g))
            si = 0
            for fut in all_slots[0:2]:
                for need in group_needs(*fut):
                    require(need)

            def emit_drain(hp_d, qb_d, cps_d):
                for par in range(2):
                    po = par * 64
                    # NB: the custom-DVE reciprocal misreads on HW when the
                    # input base partition differs from the output's, so the
                    # l row is staged to partition 0 first - on the ACT
                    # engine, which has slack here, not the busy DVE.
                    lt = lpool.tile([1, QB], F32, tag="lt", name="lt")
                    lr = lpool.tile([1, QB], F32, tag="lr", name="lr")
                    bc = lpool.tile([64, QB], F32, tag="bc", name="bc")
                    nc.vector.tensor_copy(lt, cps_d[par][64:65, :])
                    nc.vector.reciprocal_approx_fast(out=lr, in_=lt)
                    nc.gpsimd.partition_broadcast(bc, lr)
                    nc.vector.tensor_mul(
                        ctxt[
                            po : po + 64,
                            hp_d * S + qb_d * QB : hp_d * S + (qb_d + 1) * QB,
                        ],
                        cps_d[par][0:64, :],
                        bc,
                    )
                nc.sync.dma_start(
                    out=out_d[
                        hp_d * 128 : (hp_d + 1) * 128, qb_d * QB : (qb_d + 1) * QB
                    ],
                    in_=ctxt[:, hp_d * S + qb_d * QB : hp_d * S + (qb_d + 1) * QB],
                )

            pending_drain = None
            for hp in range(HPC // 2):
                for qb in range(NQB):
                    gs = groups[(hp, qb)]
                    n_units = n_units_for(hp, qb)
                    cps = {}
                    for par in range(2):
                        cps[par] = ctxps.tile([65, QB], F32, tag="cps", name=f"cps{par}")
                    unit = 0
                    prev = None
                    for gi, g in enumerate(gs):
                        for need in scores_needs(hp, qb, g):
                            require(need)
                        esb, gq0, path = emit_scores_group(hp, qb, g, gi)
                        if gi == 0 and pending_drain is not None:
                            emit_drain(*pending_drain)
                            pending_drain = None
                        for fut in all_slots[si + 1 : si + 7]:
                            for need in group_needs(*fut):
                                require_soon(need)
                        leisure_pop()
                        si += 1
                        if prev is not None:
                            for kc in prev[0]:
                                require(("v", kc))
                            unit = emit_ctx_group(hp, qb, *prev, cps, unit, n_units)
                        prev = (g, gq0, path, esb)
                    leisure_pop()
                    for kc in prev[0]:
                        require(("v", kc))
                    unit = emit_ctx_group(hp, qb, *prev, cps, unit, n_units)
                    pending_drain = (hp, qb, cps)
            flush_halves()
            emit_drain(*pending_drain)

    nc.finalize()
    return nc


def _core_inputs(x, segment_ids, Wq, bq, Wk, bk, Wv, bv, cs, core):
    b, h0 = core // 2, HPC * (core % 2)
    cols = slice(h0 * D, (h0 + HPC) * D)
    np_fp8 = mybir.dt.np(FP8)
    xT = np.ascontiguousarray(x[b].T)  # [768, 2048]
    xT_s = xT.reshape(FKC, 128, S).transpose(1, 0, 2).reshape(128, FKC * S)
    xT_bf = xT_s.reshape(128, FKC, S)[:, :, 0 : XBQ * QB].reshape(
        128, FKC * XBQ * QB
    )

    def wprep_mc(Wm):
        ws = Wm[:, cols]
        arr = ws.reshape(FKC, 128, MC, 128).transpose(1, 2, 0, 3)
        return np.ascontiguousarray(arr.reshape(128, MC * MB))

    def wprep_kc(Wm):
        ws = Wm[:, cols]
        arr = ws.reshape(FKC, 128, HD).transpose(1, 0, 2)
        return np.ascontiguousarray(arr.reshape(128, FKC * HD))

    bq_s = np.ascontiguousarray(bq[cols].reshape(MC, 128).T)
    bk_s = np.ascontiguousarray(bk[cols].reshape(MC, 128).T)
    bqk = np.concatenate([bq_s, bk_s], axis=1)
    bvb = np.ascontiguousarray(np.broadcast_to(bv[cols], (128, HD)))
    csf = cs[b].astype(np.float32)
    cs_bcast = np.ascontiguousarray(np.broadcast_to(csf, (128, S)))
    cs_part = np.ascontiguousarray(csf.reshape(NKC, KC).T)
    wq_mc = wprep_mc(Wq)
    wk_mc = wprep_mc(Wk)
    wv_kc = wprep_kc(Wv)
    return {
        "xT": np.ascontiguousarray(xT_bf).astype(ml_dtypes.bfloat16),
        "xT8": xT_s.astype(np_fp8),
        "wq16": wq_mc.astype(ml_dtypes.bfloat16),
        "wk16": wk_mc.astype(ml_dtypes.bfloat16),
        "wq8": wq_mc.astype(np_fp8),
        "wk8": wk_mc.astype(np_fp8),
        "wv16": wv_kc.astype(ml_dtypes.bfloat16),
        "wv8": wv_kc.astype(np_fp8),
        "bqk": np.ascontiguousarray(bqk),
        "bvb": bvb,
        "cs_bcast": cs_bcast,
        "cs_part": cs_part,
    }


def kernel(x, segment_ids, Wq, bq, Wk, bk, Wv, bv):
    global LAST_RESULTS
    x = np.asarray(x, np.float32)
    segment_ids = np.asarray(segment_ids)
    Wq, bq = np.asarray(Wq, np.float32), np.asarray(bq, np.float32)
    Wk, bk = np.asarray(Wk, np.float32), np.asarray(bk, np.float32)
    Wv, bv = np.asarray(Wv, np.float32), np.asarray(bv, np.float32)

    cs, vis_lists, bnd_index, qmaps = _classify(segment_ids)
    nc = _build_program(vis_lists, bnd_index, qmaps)
    in_maps = [
        _core_inputs(x, segment_ids, Wq, bq, Wk, bk, Wv, bv, cs, c)
        for c in range(NCORES)
    ]
    if TRACE:
        _ensure_ntff_hook()
    res = run_bass_kernel_spmd(nc, in_maps, list(range(NCORES)), trace=TRACE)
    LAST_RESULTS = res

    out = np.empty((B, S, W), np.float32)
    for c in range(NCORES):
        b, h0 = c // 2, HPC * (c % 2)
        out[b, :, h0 * D : (h0 + HPC) * D] = res.results[c]["ctxT"].T
    return out


# revision 63
# speedup vs baseline: 1.2513x; 1.2513x over previous
"""Trainium2 Bass kernel for nn_AttentionLayer (B=4, S=2048, H=12, D=64).

Sharding: 8 cores = 4 batches x 2 head-groups (6 heads each).
Per core: QKV projections for its 384 W-columns, then per-(head) attention
with a UniLM prefix "staircase" mask.  Fully-masked [128k x 512q] tiles are
skipped at program-build time (union over the 4 batches); partially-masked
tiles get a multiplicative 0/1 mask after exp, and only the visible
q-suffix is computed.

v3 structure (why it is shaped this way):
- The PE is output-column-bound (1 psum col/cycle @2.4GHz) regardless of
  dtype; fp8 DoubleRow's win is CONTRACTION DEPTH (256 rows/pass), not
  speed.  So: scores stay bf16 (64-deep, nothing to gain), while the
  q/k/v projections (768-deep) run as fp8e4 DoubleRow over feature-chunk
  pairs - half the passes.  ctx keeps the baseline fp8 DoubleRow k-chunk
  pairing.  Quanta feeding the error-sensitive qb=0 block (peaked
  queries that set the global max) stay bf16.
- exp is the other wall (ACT ~1 elem/cycle/partition @1.2GHz).  A
  Schraudolph DVE exp path exists (bf16 bits = int16 round of an affine
  of the score via a bitcast view; numerically validated, rel ~4.2e-3)
  but is DISABLED: in every placement tried (global 20%, tail-only 15%)
  the DVE FIFO latency it adds to the exp->mask->ctx chains cost more
  wall clock than the ACT relief bought (the kernel is chain-paced, not
  engine-throughput-paced, at the margin).
- Projection matmuls are emitted as PE filler inside the attention loop
  (keeps PE duty high so the HAM clock gate never drops the PE to
  1.2GHz); ctx runs lag-1 behind scores; softmax normalization runs off
  the PE (fast reciprocal + GPSIMD partition broadcast).
"""

import sys

if "/opt/trn_rl_repo" not in sys.path:
    sys.path.insert(0, "/opt/trn_rl_repo")

from contextlib import ExitStack

import ml_dtypes
import numpy as np

import concourse.bass as bass
import concourse.mybir as mybir
import concourse.tile as tile
from concourse import bacc, library_config
from concourse.bass_utils import run_bass_kernel_spmd

B, S, W, H, D = 4, 2048, 768, 12, 64
NCORES = 8
HPC = 6  # heads per core
QB = 512  # q block (free dim of a scores tile)
KC = 128  # k chunk (partition dim of a scores tile)
NQB = S // QB
NKC = S // KC
MC = 3  # 128-row chunks of the 384 per-core W-columns
FKC = W // 128  # feature chunks (contraction for projections)
HD = HPC * D  # 384
MB = FKC * 128  # one mc block of wq/wk columns
VE = 80  # per-head pitch in v_aug (64 d + 1 ones + pad to 16B for DoubleRow)
VW = HPC * VE  # v_aug row width per k-chunk
VW16 = HPC * (D + 1)
ACT_GROUP = 2  # k-chunks per ACT instruction = one fp8 DoubleRow ctx pair

F32 = mybir.dt.float32
BF16 = mybir.dt.bfloat16
FP8 = mybir.dt.float8e4
I16 = mybir.dt.int16
DR = mybir.MatmulPerfMode.DoubleRow

# Schraudolph exp for the DVE path: bf16 bits = round(x * 128*log2(e)/8 + b)
# (1/sqrt(D) folded into the scale; +0.5 emulates round on the truncating
# float->int16 convert).
SCH_A = 128.0 * 1.4426950408889634 / 8.0
SCH_B = 128.0 * (127.0 - 0.0450) + 0.5
DVE_EXP_FRAC = 0.0  # Schraudolph exp offload: hurt in every placement tried
DVE_HP_MIN = 2
XBQ = 2  # bf16 xt S-slices kept (nb0/nb1 only feed bf16 quanta)

TRACE = False  # set by test.py to profile
LAST_RESULTS = None  # BassKernelResults of the last run (for test.py)


def _ensure_ntff_hook():
    """This image's antenv lacks axon_hooks; register the ctypes NTFF
    profile hook from trn_agent_boot ourselves so trace=True works."""
    import types

    if "antenv.axon_hooks" in sys.modules:
        return
    try:
        from trn_agent_boot.trn_boot import _ntff_profile_via_ctypes

        hook = _ntff_profile_via_ctypes("/opt/axon/libaxon_pjrt.so")
    except Exception:
        hook = None
    mod = types.ModuleType("antenv.axon_hooks")
    mod._hook = hook
    mod.set_axon_ntff_profile_hook = lambda h: setattr(mod, "_hook", h)
    mod.get_axon_ntff_profile_hook = lambda: mod._hook
    sys.modules["antenv.axon_hooks"] = mod
    # artifact upload needs egress this sandbox doesn't have
    import concourse.bass_utils as _bu

    _bu.upload_artifacts = lambda d: "local://" + str(d)


def _classify(seg):
    """Union-over-batches tile classification from segment_ids."""
    cs = np.cumsum(np.asarray(seg, np.int64), axis=1)
    vis_lists = [[] for _ in range(NQB)]
    bnd_index = {}
    q0map = {}
    q1map = {}
    for qb in range(NQB):
        for kc in range(NKC):
            any_computed = False
            all_full_vis = True
            q0u, q1u = QB, 0
            for b in range(B):
                c = cs[b]
                full_mask = c[kc * KC] > c[qb * QB + QB - 1]
                full_vis = c[kc * KC + KC - 1] <= c[qb * QB]
                if not full_mask:
                    any_computed = True
                if not full_vis:
                    all_full_vis = False
                qcs = c[qb * QB : (qb + 1) * QB]
                anyv = np.nonzero(qcs >= c[kc * KC])[0]
                fullv = np.nonzero(qcs >= c[kc * KC + KC - 1])[0]
                q0u = min(q0u, int(anyv[0]) if len(anyv) else QB)
                q1u = max(q1u, int(fullv[0]) if len(fullv) else QB)
            if any_computed:
                vis_lists[qb].append(kc)
                if not all_full_vis:
                    bnd_index[(kc, qb)] = True
                    q0map[(kc, qb)] = (q0u // 16) * 16
                    q1map[(kc, qb)] = min(QB, ((q1u + 15) // 16) * 16)
                else:
                    q0map[(kc, qb)] = 0
                    q1map[(kc, qb)] = 0
    return cs, vis_lists, bnd_index, (q0map, q1map)


def _plan(vis_lists, bnd_index):
    """Groups, mask tile indices, and the DVE-exp group set."""
    groups = {}
    for hp in range(HPC // 2):
        for qb in range(NQB):
            vis = vis_lists[qb]
            groups[(hp, qb)] = [
                vis[i : i + ACT_GROUP] for i in range(0, len(vis), ACT_GROUP)
            ]
    mi8, mi16 = {}, {}
    for (kc, qb) in bnd_index:
        if qb == 0:
            mi16[(kc, qb)] = len(mi16)
        else:
            mi8[(kc, qb)] = len(mi8)
    total_area = 0
    cand = []
    for hp in range(HPC // 2):
        for qb in range(1, NQB):
            for gi, g in enumerate(groups[(hp, qb)]):
                a = len(g) * QB * KC
                total_area += a
                if hp >= DVE_HP_MIN and all(
                    (kc, qb) not in bnd_index for kc in g
                ):
                    cand.append((hp, qb, gi, a))
    dve_groups = set()
    target = DVE_EXP_FRAC * total_area
    acc = 0
    for hp, qb, gi, a in sorted(cand, key=lambda t: (t[2], t[0], t[1])):
        if acc >= target:
            break
        dve_groups.add((hp, qb, gi))
        acc += a
    return groups, mi8, mi16, dve_groups


def _build_program(vis_lists, bnd_index, qmaps):
    nc = bacc.Bacc()
    q0map, q1map = qmaps
    groups, mi8, mi16, dve_groups = _plan(vis_lists, bnd_index)
    N0 = len(vis_lists[0])
    assert max(vis_lists[0]) < 8, "qb0 visible chunks must sit in k nb0/nb1"
    n_bnd8 = max(len(mi8), 1)
    n_bnd16 = max(len(mi16), 1)
    vb_chunks = set(range(N0))
    for (hp, qb, gi) in dve_groups:
        vb_chunks.update(groups[(hp, qb)][gi])

    XBW = XBQ * QB  # bf16 xt pitch per feature chunk
    xT_d = nc.declare_dram_parameter("xT", [128, FKC * XBW], BF16, isOutput=False)
    xT8_d = nc.declare_dram_parameter("xT8", [128, FKC * S], FP8, isOutput=False)
    wq16_d = nc.declare_dram_parameter("wq16", [128, MC * MB], BF16, isOutput=False)
    wk16_d = nc.declare_dram_parameter("wk16", [128, MC * MB], BF16, isOutput=False)
    wq8_d = nc.declare_dram_parameter("wq8", [128, MC * MB], FP8, isOutput=False)
    wk8_d = nc.declare_dram_parameter("wk8", [128, MC * MB], FP8, isOutput=False)
    wv16_d = nc.declare_dram_parameter("wv16", [128, FKC * HD], BF16, isOutput=False)
    wv8_d = nc.declare_dram_parameter("wv8", [128, FKC * HD], FP8, isOutput=False)
    bqk_d = nc.declare_dram_parameter("bqk", [128, 2 * MC], F32, isOutput=False)
    bvb_d = nc.declare_dram_parameter("bvb", [128, HD], F32, isOutput=False)
    csb_d = nc.declare_dram_parameter("cs_bcast", [128, S], F32, isOutput=False)
    csp_d = nc.declare_dram_parameter("cs_part", [128, NKC], F32, isOutput=False)
    out_d = nc.declare_dram_parameter("ctxT", [MC * 128, S], F32, isOutput=True)

    with ExitStack() as ctx:
        tc = ctx.enter_context(tile.TileContext(nc))
        persist = ctx.enter_context(tc.tile_pool(name="persist", bufs=1))

        qt = persist.tile([128, MC * S], BF16)
        kt = persist.tile([128, MC * S], BF16)
        v = persist.tile([128, NKC * VW], FP8)
        vb16 = persist.tile([128, NKC * VW16], BF16)
        ctxt = persist.tile([128, MC * S], F32)
        msk = persist.tile([128, n_bnd8 * QB], FP8)
        mskb = persist.tile([128, n_bnd16 * QB], BF16)
        cs_b = persist.tile([128, S], F32)
        cs_p = persist.tile([128, NKC], F32)
        bqk_sb = persist.tile([128, 2 * MC], F32)
        bv_sb = persist.tile([128, HD], F32)
        warmsrc = persist.tile([128, 640], BF16)
        nc.vector.memset(warmsrc, 0.0)
        nc.gpsimd.load_library(library_config.attn)  # partition_broadcast ucode

        with (
            tc.tile_pool(name="ld", bufs=1) as ld,
            tc.tile_pool(name="pps", bufs=2, space="PSUM") as pps,
            tc.tile_pool(name="scps", bufs=2, space="PSUM") as scps,
            tc.tile_pool(name="ctxps", bufs=2, space="PSUM") as ctxps,
            tc.tile_pool(name="expp", bufs=6) as expp,
            tc.tile_pool(name="lpool", bufs=4) as lpool,
        ):
            xt = ld.tile([128, FKC * XBW], BF16)
            xt8 = ld.tile([128, FKC * S], FP8)
            wq16_sb = ld.tile([128, MC * MB], BF16)
            wk16_sb = ld.tile([128, MC * MB], BF16)
            wq8_sb = ld.tile([128, MC * MB], FP8)
            wk8_sb = ld.tile([128, MC * MB], FP8)
            wv16_sb = ld.tile([128, FKC * HD], BF16)
            wv8_sb = ld.tile([128, FKC * HD], FP8)
            # load order = first-use order: mc0 bf16 weights + x feed the
            # prelude quanta; wv16 feeds the qb0 v chunks (slots 1-2); cs
            # feeds the first boundary masks; fp8 copies are needed from
            # slot ~3 on; mc1/2 bf16 weights not until hp=1.
            # Sync carries the critical prologue chain (its queue is free at
            # t=0; the Scalar queue starts with ~2.6us of engine/act-table
            # loads).  The fp8 copies ride the Scalar queue (not needed until
            # slot ~3), cs_b rides GPSIMD behind load_library.
            # xt stays per-chunk: the prelude quantum matmuls pipeline with
            # the chunk arrivals (a single merged xt DMA makes the first
            # matmul wait the whole 1.57MB transfer: first exp +12us).  xt8
            # is merged below - not latency-critical, and it frees 5 Sync
            # issue slots.
            nc.sync.dma_start(out=wq16_sb[:, 0:MB], in_=wq16_d[:, 0:MB])
            for kc in range(3):
                nc.sync.dma_start(
                    out=xt[:, kc * XBW : (kc + 1) * XBW],
                    in_=xT_d[:, kc * XBW : (kc + 1) * XBW],
                )
            nc.sync.dma_start(out=bqk_sb, in_=bqk_d[:])
            for kc in range(3, FKC):
                nc.sync.dma_start(
                    out=xt[:, kc * XBW : (kc + 1) * XBW],
                    in_=xT_d[:, kc * XBW : (kc + 1) * XBW],
                )
            nc.sync.dma_start(out=wk16_sb[:, 0:MB], in_=wk16_d[:, 0:MB])
            # cs_b must NOT go through the GPSIMD sequencer: its DMA issue
            # doesn't fire until ~15.7us there (hidden Pool-DGE latency),
            # the first mask build then blocks the in-order DVE queue, the
            # v-projection drains behind it stall the pps psum ring, and
            # the PE sits idle 24.7-30.1us -> HAM half-clock window.
            nc.sync.dma_start(out=cs_b, in_=csb_d[:])
            nc.sync.dma_start(out=wv16_sb, in_=wv16_d[:])
            nc.sync.dma_start(out=cs_p, in_=csp_d[:])
            nc.sync.dma_start(out=bv_sb, in_=bvb_d[:])
            nc.sync.dma_start(out=wq8_sb, in_=wq8_d[:])
            nc.sync.dma_start(out=wk8_sb, in_=wk8_d[:])
            nc.sync.dma_start(out=xt8, in_=xT8_d[:])
            nc.sync.dma_start(out=wv8_sb, in_=wv8_d[:])
            nc.sync.dma_start(out=wq16_sb[:, MB:], in_=wq16_d[:, MB:])
            nc.sync.dma_start(out=wk16_sb[:, MB:], in_=wk16_d[:, MB:])

            xt84 = xt8.rearrange("p (k s) -> p k s", k=FKC)
            wq84 = wq8_sb.rearrange("p (m k c) -> p m k c", m=MC, k=FKC)
            wk84 = wk8_sb.rearrange("p (m k c) -> p m k c", m=MC, k=FKC)
            wv84 = wv8_sb.rearrange("p (k c) -> p k c", k=FKC)

            # masks are built lazily (first use) so the DVE isn't tied up
            # during the prologue while the first qk drains are demanded
            built_masks = set()

            def mask_jit(kc, qb):
                if (kc, qb) in built_masks:
                    return
                built_masks.add((kc, qb))
                if qb == 0:
                    bi, dstm = mi16[(kc, qb)], mskb
                else:
                    bi, dstm = mi8[(kc, qb)], msk
                nc.vector.tensor_scalar(
                    out=dstm[:, bi * QB : (bi + 1) * QB],
                    in0=cs_b[:, qb * QB : (qb + 1) * QB],
                    scalar1=cs_p[:, kc : kc + 1],
                    scalar2=None,
                    op0=mybir.AluOpType.is_ge,
                )

            v4 = v.rearrange("p (s h e) -> p s h e", h=HPC, e=VE)
            nc.vector.memset(v4[:, :, :, D : D + 1], 1.0)
            vb4 = vb16.rearrange("p (s h e) -> p s h e", h=HPC, e=D + 1)
            nc.vector.memset(vb4[:, :, :, D : D + 1], 1.0)
            # pre-touch the fp8 esb ring: stale regions below a split exp's
            # q0 are zeroed by the boundary mask, and 0*NaN would poison ctx
            for i in range(6):
                t8 = expp.tile([128, ACT_GROUP * QB], FP8, tag="esb8", name="z8")
                nc.gpsimd.memset(t8, 0.0)

            # --- projection quanta ---------------------------------------
            # bf16 quanta are 6 matmuls (~1.3us) - the largest PE lump.  When
            # popped as leisure filler they are emitted as two 3-matmul
            # halves across consecutive pops, so the PE is never occupied for
            # a full quantum right when a scores psum buffer releases (that
            # jitter is what opens the ~0.5us/slot gaps in the exp stream).
            half_pending = {}  # fq -> psum tile awaiting kc3-5 + drain

            def _qk_bf16_mms(ps, pi, mc, nb, lo, hi):
                w_sb = wq16_sb if pi == 0 else wk16_sb
                for kc in range(lo, hi):
                    nc.tensor.matmul(
                        ps,
                        lhsT=w_sb[:, mc * MB + kc * 128 : mc * MB + kc * 128 + 128],
                        rhs=xt[:, kc * XBW + nb * QB : kc * XBW + (nb + 1) * QB],
                        start=(kc == 0),
                        stop=(kc == FKC - 1),
                    )

            def _qk_drain(ps, pi, mc, nb, drain_on_act):
                out_sb = qt if pi == 0 else kt
                if drain_on_act:
                    nc.scalar.activation(
                        out=out_sb[:, mc * S + nb * QB : mc * S + (nb + 1) * QB],
                        in_=ps,
                        func=mybir.ActivationFunctionType.Identity,
                        bias=bqk_sb[:, pi * MC + mc : pi * MC + mc + 1],
                        scale=1.0,
                    )
                else:
                    nc.vector.tensor_scalar_add(
                        out_sb[:, mc * S + nb * QB : mc * S + (nb + 1) * QB],
                        ps,
                        bqk_sb[:, pi * MC + mc : pi * MC + mc + 1],
                    )

            def finish_half(fq, drain_on_act=True):
                ps = half_pending.pop(fq)
                _qk_bf16_mms(ps, fq[1], fq[2], fq[3], FKC // 2, FKC)
                _qk_drain(ps, fq[1], fq[2], fq[3], drain_on_act)

            def qk_quantum(pi, mc, nb, drain_on_act=False, first_half_only=False):
                kind_bf16 = (pi == 0 and nb == 0) or (pi == 1 and nb <= 1)
                ps = pps.tile([128, QB], F32, tag="proj", name="psqk")
                if kind_bf16:
                    if first_half_only:
                        _qk_bf16_mms(ps, pi, mc, nb, 0, FKC // 2)
                        half_pending[("qk", pi, mc, nb)] = ps
                        return
                    _qk_bf16_mms(ps, pi, mc, nb, 0, FKC)
                else:
                    w4 = wq84 if pi == 0 else wk84
                    for j in range(FKC // 2):
                        nc.tensor.matmul(
                            ps,
                            lhsT=w4[:, mc, 2 * j : 2 * j + 2, :],
                            rhs=xt84[:, 2 * j : 2 * j + 2, nb * QB : (nb + 1) * QB],
                            start=(j == 0),
                            stop=(j == FKC // 2 - 1),
                            perf_mode=DR,
                        )
                _qk_drain(ps, pi, mc, nb, drain_on_act)

            def v_quantum(sc):
                ps = pps.tile([128, HD], F32, tag="proj", name="psv")
                if sc < N0:
                    for kc in range(FKC):
                        nc.tensor.matmul(
                            ps,
                            lhsT=xt[:, kc * XBW + sc * KC : kc * XBW + sc * KC + KC],
                            rhs=wv16_sb[:, kc * HD : (kc + 1) * HD],
                            start=(kc == 0),
                            stop=(kc == FKC - 1),
                        )
                else:
                    for j in range(FKC // 2):
                        nc.tensor.matmul(
                            ps,
                            lhsT=xt84[:, 2 * j : 2 * j + 2, sc * KC : sc * KC + KC],
                            rhs=wv84[:, 2 * j : 2 * j + 2, :],
                            start=(j == 0),
                            stop=(j == FKC // 2 - 1),
                            perf_mode=DR,
                        )
                nc.vector.tensor_add(
                    v4[:, sc, :, 0:D],
                    ps.rearrange("p (h e) -> p h e", e=D),
                    bv_sb.rearrange("p (h e) -> p h e", e=D),
                )
                if sc in vb_chunks:
                    nc.vector.tensor_add(
                        vb4[:, sc, :, 0:D],
                        ps.rearrange("p (h e) -> p h e", e=D),
                        bv_sb.rearrange("p (h e) -> p h e", e=D),
                    )

            emitted = set()

            def flush_halves(drain_on_act=True):
                # a pending half holds a pps ring buffer; its completing
                # matmuls+drain must precede any further pps allocation in
                # the in-order PE FIFO or a later allocation can deadlock
                for f in list(half_pending):
                    finish_half(f, drain_on_act)
                    emitted.add(f)

            def emit_quantum(fq, drain_on_act=False, first_half=False):
                if fq in emitted or fq in half_pending:
                    return
                flush_halves()
                kind_bf16 = fq[0] == "qk" and (
                    (fq[1] == 0 and fq[3] == 0) or (fq[1] == 1 and fq[3] <= 1)
                )
                if fq[0] == "v":
                    emitted.add(fq)
                    v_quantum(fq[1])
                    return
                if first_half and kind_bf16:
                    qk_quantum(fq[1], fq[2], fq[3], drain_on_act, first_half_only=True)
                    return
                emitted.add(fq)
                qk_quantum(fq[1], fq[2], fq[3], drain_on_act)

            # filler order: mc0 fp8 quanta, then the HEAVY bf16 quanta of
            # mc1/mc2 spread early (so the hp transitions never demand a
            # burst of 1.3us quanta at once), then v tail, then fp8 mc1/2.
            filler = []
            for nb in range(1, NQB):
                filler.append(("qk", 0, 0, nb))
            for nb in range(2, NQB):
                filler.append(("qk", 1, 0, nb))
            for sc in range(N0, N0 + 4):
                filler.append(("v", sc))
            for mc in (1, 2):
                filler.append(("qk", 0, mc, 0))
                filler.append(("qk", 1, mc, 0))
                filler.append(("qk", 1, mc, 1))
            for sc in range(N0 + 4, NKC):
                filler.append(("v", sc))
            for mc in (1, 2):
                for pi in range(2):
                    for nb in range(1 if pi == 0 else 2, NQB):
                        filler.append(("qk", pi, mc, nb))

            demand_q = []

            def require(fq):
                if fq in emitted:
                    return
                if fq in half_pending:
                    finish_half(fq, drain_on_act=False)  # demanded: DVE drain
                    emitted.add(fq)
                    return
                if fq in filler:
                    filler.remove(fq)
                if fq in demand_q:
                    demand_q.remove(fq)
                emit_quantum(fq)

            def require_soon(fq):
                if fq in emitted or fq in half_pending or fq in demand_q:
                    return
                if fq in filler:
                    filler.remove(fq)
                demand_q.append(fq)

            def warm_dummy():
                ps = pps.tile([128, QB], F32, tag="proj", name="warm")
                nc.tensor.matmul(
                    ps,
                    lhsT=warmsrc[:, 0:128],
                    rhs=warmsrc[:, 128 : 128 + QB],
                    start=True,
                    stop=True,
                )

            def leisure_pop():
                if half_pending:
                    flush_halves()
                elif demand_q:
                    emit_quantum(demand_q.pop(0))
                elif filler:
                    # NB: half-quantum smoothing (first_half=True) tested
                    # 3.3us WORSE than whole-quantum pops; keep whole.
                    emit_quantum(filler.pop(0), drain_on_act=True)
                else:
                    for _ in range(4):
                        warm_dummy()

            def scores_needs(hp, qb, g):
                needs = [("qk", 0, hp, qb)]
                nb_hi = (g[-1] * KC + KC - 1) // QB
                for nb in range(nb_hi + 1):
                    needs.append(("qk", 1, hp, nb))
                return needs

            def group_needs(hp, qb, g):
                return scores_needs(hp, qb, g) + [("v", kc) for kc in g]

            emit_quantum(("qk", 0, 0, 0))
            emit_quantum(("qk", 1, 0, 0))
            for sc in range(N0):
                filler.insert(sc, ("v", sc))

            # --- attention -----------------------------------------------
            def emit_scores_group(hp, qb, g, gi):
                if qb == 0:
                    path = "bf16"
                elif (hp, qb, gi) in dve_groups:
                    path = "dve"
                else:
                    path = "fp8"
                q0s = [q0map[(kc, qb)] for kc in g]
                gq0 = min(q0s)
                # per-chunk exp ranges only on the fp8 path (its esb ring is
                # pre-zeroed; stale [gq0,q0c) is masked to 0, never NaN)
                exp_split = (max(q0s) - gq0) >= 224 and path == "fp8"
                eq0 = [q0 if (exp_split or q0 == gq0) else gq0 for q0 in q0s]

                for kc in g:
                    if (kc, qb) in bnd_index:
                        mask_jit(kc, qb)
                mcq = hp
                sps = {}
                esb = {}
                etag = {"bf16": "esb16b", "fp8": "esb8", "dve": "esb16"}[path]
                edt = FP8 if path == "fp8" else BF16
                for par in range(2):
                    sps[par] = scps.tile(
                        [128, ACT_GROUP * QB], F32, tag="sps", name=f"sps{par}"
                    )
                    esb[par] = expp.tile(
                        [128, ACT_GROUP * QB], edt, tag=etag, name=f"esb{par}"
                    )
                # par-outer: par0's scores finish 1 matmul earlier, so its
                # exp (the slot's pacing chain) starts sooner; each par's
                # masks follow its own exp so ctx(par0) is ready while
                # exp(par1) still runs.
                scale = 1.0 / float(np.sqrt(np.float32(D)))
                for par in range(2):
                    po = par * 64
                    for j, kc in enumerate(g):
                        nc.tensor.matmul(
                            sps[par][:, j * QB + eq0[j] : (j + 1) * QB],
                            lhsT=kt[
                                po : po + 64, mcq * S + kc * KC : mcq * S + kc * KC + KC
                            ],
                            rhs=qt[
                                po : po + 64,
                                mcq * S + qb * QB + eq0[j] : mcq * S + (qb + 1) * QB,
                            ],
                            start=True,
                            stop=True,
                        )
                for par in range(2):
                    if path == "dve":
                        nc.vector.tensor_scalar(
                            out=esb[par][:, 0 : len(g) * QB].bitcast(I16),
                            in0=sps[par][:, 0 : len(g) * QB],
                            scalar1=SCH_A,
                            scalar2=SCH_B,
                            op0=mybir.AluOpType.mult,
                            op1=mybir.AluOpType.add,
                        )
                    elif exp_split or len(g) == 1:
                        for j in range(len(g)):
                            nc.scalar.activation(
                                out=esb[par][:, j * QB + eq0[j] : (j + 1) * QB],
                                in_=sps[par][:, j * QB + eq0[j] : (j + 1) * QB],
                                func=mybir.ActivationFunctionType.Exp,
                                scale=scale,
                            )
                    else:
                        src = sps[par].rearrange("p (j q) -> p j q", j=2)[:, :, gq0:]
                        dst = esb[par].rearrange("p (j q) -> p j q", j=2)[:, :, gq0:]
                        nc.scalar.activation(
                            out=dst,
                            in_=src,
                            func=mybir.ActivationFunctionType.Exp,
                            scale=scale,
                        )
                    for j, kc in enumerate(g):
                        if (kc, qb) not in bnd_index:
                            continue
                        q1 = q1map[(kc, qb)]
                        if qb == 0:
                            bi, srcm = mi16[(kc, qb)], mskb
                        else:
                            bi, srcm = mi8[(kc, qb)], msk
                        nc.vector.tensor_mul(
                            esb[par][:, j * QB + gq0 : j * QB + q1],
                            esb[par][:, j * QB + gq0 : j * QB + q1],
                            srcm[:, bi * QB + gq0 : bi * QB + q1],
                        )
                return esb, gq0, path

            def emit_ctx_group(hp, qb, g, gq0, path, esb, cps, unit, n_units):
                if path == "fp8":
                    for par in range(2):
                        h = 2 * hp + par
                        if len(g) == 2:
                            nc.tensor.matmul(
                                cps[par][:, gq0:],
                                lhsT=v4[:, g[0] : g[0] + 2, h, 0 : D + 1],
                                rhs=esb[par].rearrange("p (j q) -> p j q", j=2)[
                                    :, :, gq0:
                                ],
                                start=(unit == 0),
                                stop=(unit == n_units - 1),
                                perf_mode=DR,
                            )
                        else:
                            nc.tensor.matmul(
                                cps[par][:, gq0:],
                                lhsT=v4[:, g[0], h, 0 : D + 1],
                                rhs=esb[par][:, gq0:QB],
                                start=(unit == 0),
                                stop=(unit == n_units - 1),
                            )
                    return unit + 1
                for j, kc in enumerate(g):
                    for par in range(2):
                        h = 2 * hp + par
                        nc.tensor.matmul(
                            cps[par][:, gq0:],
                            lhsT=vb16[
                                :, kc * VW16 + h * (D + 1) : kc * VW16 + (h + 1) * (D + 1)
                            ],
                            rhs=esb[par][:, j * QB + gq0 : (j + 1) * QB],
                            start=(unit + j == 0),
                            stop=(unit + j == n_units - 1),
                        )
                return unit + len(g)

            def n_units_for(hp, qb):
                n = 0
                for gi, g in enumerate(groups[(hp, qb)]):
                    if qb == 0 or (hp, qb, gi) in dve_groups:
                        n += len(g)
                    else:
                        n += 1
                return n

            all_slots = []
            for hp in range(HPC // 2):
                for qb in range(NQB):
                    for gi, g in enumerate(groups[(hp, qb)]):
                        all_slots.append((hp, qb, g))
            si = 0
            for fut in all_slots[0:2]:
                for need in group_needs(*fut):
                    require(need)

            def emit_drain(hp_d, qb_d, cps_d):
                for par in range(2):
                    po = par * 64
                    # NB: the custom-DVE reciprocal misreads on HW when the
                    # input base partition differs from the output's, so the
                    # l row is staged to partition 0 first - on the ACT
                    # engine, which has slack here, not the busy DVE.
                    lt = lpool.tile([1, QB], F32, tag="lt", name="lt")
                    lr = lpool.tile([1, QB], F32, tag="lr", name="lr")
                    bc = lpool.tile([64, QB], F32, tag="bc", name="bc")
                    nc.vector.tensor_copy(lt, cps_d[par][64:65, :])
                    nc.vector.reciprocal_approx_fast(out=lr, in_=lt)
                    nc.gpsimd.partition_broadcast(bc, lr)
                    nc.vector.tensor_mul(
                        ctxt[
                            po : po + 64,
                            hp_d * S + qb_d * QB : hp_d * S + (qb_d + 1) * QB,
                        ],
                        cps_d[par][0:64, :],
                        bc,
                    )
                nc.sync.dma_start(
                    out=out_d[
                        hp_d * 128 : (hp_d + 1) * 128, qb_d * QB : (qb_d + 1) * QB
                    ],
                    in_=ctxt[:, hp_d * S + qb_d * QB : hp_d * S + (qb_d + 1) * QB],
                )

            pending_drain = None
            for hp in range(HPC // 2):
                for qb in range(NQB):
                    gs = groups[(hp, qb)]
                    n_units = n_units_for(hp, qb)
                    cps = {}
                    for par in range(2):
                        cps[par] = ctxps.tile([65, QB], F32, tag="cps", name=f"cps{par}")
                    unit = 0
                    prev = None
                    for gi, g in enumerate(gs):
                        for need in scores_needs(hp, qb, g):
                            require(need)
                        esb, gq0, path = emit_scores_group(hp, qb, g, gi)
                        if gi == 0 and pending_drain is not None:
                            emit_drain(*pending_drain)
                            pending_drain = None
                        for fut in all_slots[si + 1 : si + 7]:
                            for need in group_needs(*fut):
                                require_soon(need)
                        leisure_pop()
                        si += 1
                        if prev is not None:
                            for kc in prev[0]:
                                require(("v", kc))
                            unit = emit_ctx_group(hp, qb, *prev, cps, unit, n_units)
                        prev = (g, gq0, path, esb)
                    leisure_pop()
                    for kc in prev[0]:
                        require(("v", kc))
                    unit = emit_ctx_group(hp, qb, *prev, cps, unit, n_units)
                    pending_drain = (hp, qb, cps)
            flush_halves()
            emit_drain(*pending_drain)

    nc.finalize()
    return nc


def _core_inputs(x, segment_ids, Wq, bq, Wk, bk, Wv, bv, cs, core):
    b, h0 = core // 2, HPC * (core % 2)
    cols = slice(h0 * D, (h0 + HPC) * D)
    np_fp8 = mybir.dt.np(FP8)
    xT = np.ascontiguousarray(x[b].T)  # [768, 2048]
    xT_s = xT.reshape(FKC, 128, S).transpose(1, 0, 2).reshape(128, FKC * S)
    xT_bf = xT_s.reshape(128, FKC, S)[:, :, 0 : XBQ * QB].reshape(
        128, FKC * XBQ * QB
    )

    def wprep_mc(Wm):
        ws = Wm[:, cols]
        arr = ws.reshape(FKC, 128, MC, 128).transpose(1, 2, 0, 3)
        return np.ascontiguousarray(arr.reshape(128, MC * MB))

    def wprep_kc(Wm):
        ws = Wm[:, cols]
        arr = ws.reshape(FKC, 128, HD).transpose(1, 0, 2)
        return np.ascontiguousarray(arr.reshape(128, FKC * HD))

    bq_s = np.ascontiguousarray(bq[cols].reshape(MC, 128).T)
    bk_s = np.ascontiguousarray(bk[cols].reshape(MC, 128).T)
    bqk = np.concatenate([bq_s, bk_s], axis=1)
    bvb = np.ascontiguousarray(np.broadcast_to(bv[cols], (128, HD)))
    csf = cs[b].astype(np.float32)
    cs_bcast = np.ascontiguousarray(np.broadcast_to(csf, (128, S)))
    cs_part = np.ascontiguousarray(csf.reshape(NKC, KC).T)
    wq_mc = wprep_mc(Wq)
    wk_mc = wprep_mc(Wk)
    wv_kc = wprep_kc(Wv)
    return {
        "xT": np.ascontiguousarray(xT_bf).astype(ml_dtypes.bfloat16),
        "xT8": xT_s.astype(np_fp8),
        "wq16": wq_mc.astype(ml_dtypes.bfloat16),
        "wk16": wk_mc.astype(ml_dtypes.bfloat16),
        "wq8": wq_mc.astype(np_fp8),
        "wk8": wk_mc.astype(np_fp8),
        "wv16": wv_kc.astype(ml_dtypes.bfloat16),
        "wv8": wv_kc.astype(np_fp8),
        "bqk": np.ascontiguousarray(bqk),
        "bvb": bvb,
        "cs_bcast": cs_bcast,
        "cs_part": cs_part,
    }


def kernel(x, segment_ids, Wq, bq, Wk, bk, Wv, bv):
    global LAST_RESULTS
    x = np.asarray(x, np.float32)
    segment_ids = np.asarray(segment_ids)
    Wq, bq = np.asarray(Wq, np.float32), np.asarray(bq, np.float32)
    Wk, bk = np.asarray(Wk, np.float32), np.asarray(bk, np.float32)
    Wv, bv = np.asarray(Wv, np.float32), np.asarray(bv, np.float32)

    cs, vis_lists, bnd_index, qmaps = _classify(segment_ids)
    nc = _build_program(vis_lists, bnd_index, qmaps)
    in_maps = [
        _core_inputs(x, segment_ids, Wq, bq, Wk, bk, Wv, bv, cs, c)
        for c in range(NCORES)
    ]
    if TRACE:
        _ensure_ntff_hook()
    res = run_bass_kernel_spmd(nc, in_maps, list(range(NCORES)), trace=TRACE)
    LAST_RESULTS = res

    out = np.empty((B, S, W), np.float32)
    for c in range(NCORES):
        b, h0 = c // 2, HPC * (c % 2)
        out[b, :, h0 * D : (h0 + HPC) * D] = res.results[c]["ctxT"].T
    return out


# revision 64
# speedup vs baseline: 1.2529x; 1.0013x over previous
"""Trainium2 Bass kernel for nn_AttentionLayer (B=4, S=2048, H=12, D=64).

Sharding: 8 cores = 4 batches x 2 head-groups (6 heads each).
Per core: QKV projections for its 384 W-columns, then per-(head) attention
with a UniLM prefix "staircase" mask.  Fully-masked [128k x 512q] tiles are
skipped at program-build time (union over the 4 batches); partially-masked
tiles get a multiplicative 0/1 mask after exp, and only the visible
q-suffix is computed.

v3 structure (why it is shaped this way):
- The PE is output-column-bound (1 psum col/cycle @2.4GHz) regardless of
  dtype; fp8 DoubleRow's win is CONTRACTION DEPTH (256 rows/pass), not
  speed.  So: scores stay bf16 (64-deep, nothing to gain), while the
  q/k/v projections (768-deep) run as fp8e4 DoubleRow over feature-chunk
  pairs - half the passes.  ctx keeps the baseline fp8 DoubleRow k-chunk
  pairing.  Quanta feeding the error-sensitive qb=0 block (peaked
  queries that set the global max) stay bf16.
- exp is the other wall (ACT ~1 elem/cycle/partition @1.2GHz).  A
  Schraudolph DVE exp path exists (bf16 bits = int16 round of an affine
  of the score via a bitcast view; numerically validated, rel ~4.2e-3)
  but is DISABLED: in every placement tried (global 20%, tail-only 15%)
  the DVE FIFO latency it adds to the exp->mask->ctx chains cost more
  wall clock than the ACT relief bought (the kernel is chain-paced, not
  engine-throughput-paced, at the margin).
- Projection matmuls are emitted as PE filler inside the attention loop
  (keeps PE duty high so the HAM clock gate never drops the PE to
  1.2GHz); ctx runs lag-1 behind scores; softmax normalization runs off
  the PE (fast reciprocal + GPSIMD partition broadcast).
"""

import sys

if "/opt/trn_rl_repo" not in sys.path:
    sys.path.insert(0, "/opt/trn_rl_repo")

from contextlib import ExitStack

import ml_dtypes
import numpy as np

import concourse.bass as bass
import concourse.mybir as mybir
import concourse.tile as tile
from concourse import bacc, library_config
from concourse.bass_utils import run_bass_kernel_spmd

B, S, W, H, D = 4, 2048, 768, 12, 64
NCORES = 8
HPC = 6  # heads per core
QB = 512  # q block (free dim of a scores tile)
KC = 128  # k chunk (partition dim of a scores tile)
NQB = S // QB
NKC = S // KC
MC = 3  # 128-row chunks of the 384 per-core W-columns
FKC = W // 128  # feature chunks (contraction for projections)
HD = HPC * D  # 384
MB = FKC * 128  # one mc block of wq/wk columns
VE = 80  # per-head pitch in v_aug (64 d + 1 ones + pad to 16B for DoubleRow)
VW = HPC * VE  # v_aug row width per k-chunk
VW16 = HPC * (D + 1)
ACT_GROUP = 2  # k-chunks per ACT instruction = one fp8 DoubleRow ctx pair

F32 = mybir.dt.float32
BF16 = mybir.dt.bfloat16
FP8 = mybir.dt.float8e4
I16 = mybir.dt.int16
DR = mybir.MatmulPerfMode.DoubleRow

# Schraudolph exp for the DVE path: bf16 bits = round(x * 128*log2(e)/8 + b)
# (1/sqrt(D) folded into the scale; +0.5 emulates round on the truncating
# float->int16 convert).
SCH_A = 128.0 * 1.4426950408889634 / 8.0
SCH_B = 128.0 * (127.0 - 0.0450) + 0.5
DVE_EXP_FRAC = 0.0  # Schraudolph exp offload: hurt in every placement tried
DVE_HP_MIN = 2
XBQ = 2  # bf16 xt S-slices kept (nb0/nb1 only feed bf16 quanta)

TRACE = False  # set by test.py to profile
LAST_RESULTS = None  # BassKernelResults of the last run (for test.py)


def _ensure_ntff_hook():
    """This image's antenv lacks axon_hooks; register the ctypes NTFF
    profile hook from trn_agent_boot ourselves so trace=True works."""
    import types

    if "antenv.axon_hooks" in sys.modules:
        return
    try:
        from trn_agent_boot.trn_boot import _ntff_profile_via_ctypes

        hook = _ntff_profile_via_ctypes("/opt/axon/libaxon_pjrt.so")
    except Exception:
        hook = None
    mod = types.ModuleType("antenv.axon_hooks")
    mod._hook = hook
    mod.set_axon_ntff_profile_hook = lambda h: setattr(mod, "_hook", h)
    mod.get_axon_ntff_profile_hook = lambda: mod._hook
    sys.modules["antenv.axon_hooks"] = mod
    # artifact upload needs egress this sandbox doesn't have
    import concourse.bass_utils as _bu

    _bu.upload_artifacts = lambda d: "local://" + str(d)


def _classify(seg):
    """Union-over-batches tile classification from segment_ids."""
    cs = np.cumsum(np.asarray(seg, np.int64), axis=1)
    vis_lists = [[] for _ in range(NQB)]
    bnd_index = {}
    q0map = {}
    q1map = {}
    for qb in range(NQB):
        for kc in range(NKC):
            any_computed = False
            all_full_vis = True
            q0u, q1u = QB, 0
            for b in range(B):
                c = cs[b]
                full_mask = c[kc * KC] > c[qb * QB + QB - 1]
                full_vis = c[kc * KC + KC - 1] <= c[qb * QB]
                if not full_mask:
                    any_computed = True
                if not full_vis:
                    all_full_vis = False
                qcs = c[qb * QB : (qb + 1) * QB]
                anyv = np.nonzero(qcs >= c[kc * KC])[0]
                fullv = np.nonzero(qcs >= c[kc * KC + KC - 1])[0]
                q0u = min(q0u, int(anyv[0]) if len(anyv) else QB)
                q1u = max(q1u, int(fullv[0]) if len(fullv) else QB)
            if any_computed:
                vis_lists[qb].append(kc)
                if not all_full_vis:
                    bnd_index[(kc, qb)] = True
                    q0map[(kc, qb)] = (q0u // 16) * 16
                    q1map[(kc, qb)] = min(QB, ((q1u + 15) // 16) * 16)
                else:
                    q0map[(kc, qb)] = 0
                    q1map[(kc, qb)] = 0
    return cs, vis_lists, bnd_index, (q0map, q1map)


def _plan(vis_lists, bnd_index):
    """Groups, mask tile indices, and the DVE-exp group set."""
    groups = {}
    for hp in range(HPC // 2):
        for qb in range(NQB):
            vis = vis_lists[qb]
            groups[(hp, qb)] = [
                vis[i : i + ACT_GROUP] for i in range(0, len(vis), ACT_GROUP)
            ]
    mi8, mi16 = {}, {}
    for (kc, qb) in bnd_index:
        if qb == 0:
            mi16[(kc, qb)] = len(mi16)
        else:
            mi8[(kc, qb)] = len(mi8)
    total_area = 0
    cand = []
    for hp in range(HPC // 2):
        for qb in range(1, NQB):
            for gi, g in enumerate(groups[(hp, qb)]):
                a = len(g) * QB * KC
                total_area += a
                if hp >= DVE_HP_MIN and all(
                    (kc, qb) not in bnd_index for kc in g
                ):
                    cand.append((hp, qb, gi, a))
    dve_groups = set()
    target = DVE_EXP_FRAC * total_area
    acc = 0
    for hp, qb, gi, a in sorted(cand, key=lambda t: (t[2], t[0], t[1])):
        if acc >= target:
            break
        dve_groups.add((hp, qb, gi))
        acc += a
    return groups, mi8, mi16, dve_groups


def _build_program(vis_lists, bnd_index, qmaps):
    nc = bacc.Bacc()
    q0map, q1map = qmaps
    groups, mi8, mi16, dve_groups = _plan(vis_lists, bnd_index)
    N0 = len(vis_lists[0])
    assert max(vis_lists[0]) < 8, "qb0 visible chunks must sit in k nb0/nb1"
    n_bnd8 = max(len(mi8), 1)
    n_bnd16 = max(len(mi16), 1)
    vb_chunks = set(range(N0))
    for (hp, qb, gi) in dve_groups:
        vb_chunks.update(groups[(hp, qb)][gi])

    XBW = XBQ * QB  # bf16 xt pitch per feature chunk
    xT_d = nc.declare_dram_parameter("xT", [128, FKC * XBW], BF16, isOutput=False)
    xT8_d = nc.declare_dram_parameter("xT8", [128, FKC * S], FP8, isOutput=False)
    wq16_d = nc.declare_dram_parameter("wq16", [128, MC * MB], BF16, isOutput=False)
    wk16_d = nc.declare_dram_parameter("wk16", [128, MC * MB], BF16, isOutput=False)
    wq8_d = nc.declare_dram_parameter("wq8", [128, MC * MB], FP8, isOutput=False)
    wk8_d = nc.declare_dram_parameter("wk8", [128, MC * MB], FP8, isOutput=False)
    wv16_d = nc.declare_dram_parameter("wv16", [128, FKC * HD], BF16, isOutput=False)
    wv8_d = nc.declare_dram_parameter("wv8", [128, FKC * HD], FP8, isOutput=False)
    bqk_d = nc.declare_dram_parameter("bqk", [128, 2 * MC], F32, isOutput=False)
    bvb_d = nc.declare_dram_parameter("bvb", [128, HD], F32, isOutput=False)
    csb_d = nc.declare_dram_parameter("cs_bcast", [128, S], F32, isOutput=False)
    csp_d = nc.declare_dram_parameter("cs_part", [128, NKC], F32, isOutput=False)
    out_d = nc.declare_dram_parameter("ctxT", [MC * 128, S], F32, isOutput=True)

    with ExitStack() as ctx:
        tc = ctx.enter_context(tile.TileContext(nc))
        persist = ctx.enter_context(tc.tile_pool(name="persist", bufs=1))

        qt = persist.tile([128, MC * S], BF16)
        kt = persist.tile([128, MC * S], BF16)
        v = persist.tile([128, NKC * VW], FP8)
        vb16 = persist.tile([128, NKC * VW16], BF16)
        ctxt = persist.tile([128, MC * S], F32)
        msk = persist.tile([128, n_bnd8 * QB], FP8)
        mskb = persist.tile([128, n_bnd16 * QB], BF16)
        cs_b = persist.tile([128, S], F32)
        cs_p = persist.tile([128, NKC], F32)
        bqk_sb = persist.tile([128, 2 * MC], F32)
        bv_sb = persist.tile([128, HD], F32)
        warmsrc = persist.tile([128, 640], BF16)
        nc.vector.memset(warmsrc, 0.0)
        nc.gpsimd.load_library(library_config.attn)  # partition_broadcast ucode

        with (
            tc.tile_pool(name="ld", bufs=1) as ld,
            tc.tile_pool(name="pps", bufs=2, space="PSUM") as pps,
            tc.tile_pool(name="scps", bufs=2, space="PSUM") as scps,
            tc.tile_pool(name="ctxps", bufs=2, space="PSUM") as ctxps,
            tc.tile_pool(name="expp", bufs=6) as expp,
            tc.tile_pool(name="lpool", bufs=4) as lpool,
        ):
            xt = ld.tile([128, FKC * XBW], BF16)
            xt8 = ld.tile([128, FKC * S], FP8)
            wq16_sb = ld.tile([128, MC * MB], BF16)
            wk16_sb = ld.tile([128, MC * MB], BF16)
            wq8_sb = ld.tile([128, MC * MB], FP8)
            wk8_sb = ld.tile([128, MC * MB], FP8)
            wv16_sb = ld.tile([128, FKC * HD], BF16)
            wv8_sb = ld.tile([128, FKC * HD], FP8)
            # load order = first-use order: mc0 bf16 weights + x feed the
            # prelude quanta; wv16 feeds the qb0 v chunks (slots 1-2); cs
            # feeds the first boundary masks; fp8 copies are needed from
            # slot ~3 on; mc1/2 bf16 weights not until hp=1.
            # Sync carries the critical prologue chain (its queue is free at
            # t=0; the Scalar queue starts with ~2.6us of engine/act-table
            # loads).  The fp8 copies ride the Scalar queue (not needed until
            # slot ~3), cs_b rides GPSIMD behind load_library.
            # xt stays per-chunk: the prelude quantum matmuls pipeline with
            # the chunk arrivals (a single merged xt DMA makes the first
            # matmul wait the whole 1.57MB transfer: first exp +12us).  xt8
            # is merged below - not latency-critical, and it frees 5 Sync
            # issue slots.
            nc.sync.dma_start(out=wq16_sb[:, 0:MB], in_=wq16_d[:, 0:MB])
            for kc in range(3):
                nc.sync.dma_start(
                    out=xt[:, kc * XBW : (kc + 1) * XBW],
                    in_=xT_d[:, kc * XBW : (kc + 1) * XBW],
                )
            nc.sync.dma_start(out=bqk_sb, in_=bqk_d[:])
            for kc in range(3, FKC):
                nc.sync.dma_start(
                    out=xt[:, kc * XBW : (kc + 1) * XBW],
                    in_=xT_d[:, kc * XBW : (kc + 1) * XBW],
                )
            nc.sync.dma_start(out=wk16_sb[:, 0:MB], in_=wk16_d[:, 0:MB])
            # cs_b must NOT go through the GPSIMD sequencer: its DMA issue
            # doesn't fire until ~15.7us there (hidden Pool-DGE latency),
            # the first mask build then blocks the in-order DVE queue, the
            # v-projection drains behind it stall the pps psum ring, and
            # the PE sits idle 24.7-30.1us -> HAM half-clock window.
            nc.sync.dma_start(out=cs_b, in_=csb_d[:])
            nc.sync.dma_start(out=wv16_sb, in_=wv16_d[:])
            nc.sync.dma_start(out=cs_p, in_=csp_d[:])
            nc.sync.dma_start(out=bv_sb, in_=bvb_d[:])
            nc.sync.dma_start(out=wq8_sb, in_=wq8_d[:])
            nc.sync.dma_start(out=wk8_sb, in_=wk8_d[:])
            nc.sync.dma_start(out=xt8, in_=xT8_d[:])
            nc.sync.dma_start(out=wv8_sb, in_=wv8_d[:])
            nc.sync.dma_start(out=wq16_sb[:, MB:], in_=wq16_d[:, MB:])
            nc.sync.dma_start(out=wk16_sb[:, MB:], in_=wk16_d[:, MB:])

            xt84 = xt8.rearrange("p (k s) -> p k s", k=FKC)
            wq84 = wq8_sb.rearrange("p (m k c) -> p m k c", m=MC, k=FKC)
            wk84 = wk8_sb.rearrange("p (m k c) -> p m k c", m=MC, k=FKC)
            wv84 = wv8_sb.rearrange("p (k c) -> p k c", k=FKC)

            # masks are built lazily (first use) so the DVE isn't tied up
            # during the prologue while the first qk drains are demanded
            built_masks = set()

            def mask_jit(kc, qb):
                if (kc, qb) in built_masks:
                    return
                built_masks.add((kc, qb))
                if qb == 0:
                    bi, dstm = mi16[(kc, qb)], mskb
                else:
                    bi, dstm = mi8[(kc, qb)], msk
                nc.vector.tensor_scalar(
                    out=dstm[:, bi * QB : (bi + 1) * QB],
                    in0=cs_b[:, qb * QB : (qb + 1) * QB],
                    scalar1=cs_p[:, kc : kc + 1],
                    scalar2=None,
                    op0=mybir.AluOpType.is_ge,
                )

            v4 = v.rearrange("p (s h e) -> p s h e", h=HPC, e=VE)
            nc.vector.memset(v4[:, :, :, D : D + 1], 1.0)
            vb4 = vb16.rearrange("p (s h e) -> p s h e", h=HPC, e=D + 1)
            nc.vector.memset(vb4[:, :, :, D : D + 1], 1.0)
            # pre-touch the fp8 esb ring: stale regions below a split exp's
            # q0 are zeroed by the boundary mask, and 0*NaN would poison ctx
            for i in range(6):
                t8 = expp.tile([128, ACT_GROUP * QB], FP8, tag="esb8", name="z8")
                nc.gpsimd.memset(t8, 0.0)

            # --- projection quanta ---------------------------------------
            # bf16 quanta are 6 matmuls (~1.3us) - the largest PE lump.  When
            # popped as leisure filler they are emitted as two 3-matmul
            # halves across consecutive pops, so the PE is never occupied for
            # a full quantum right when a scores psum buffer releases (that
            # jitter is what opens the ~0.5us/slot gaps in the exp stream).
            half_pending = {}  # fq -> psum tile awaiting kc3-5 + drain

            def _qk_bf16_mms(ps, pi, mc, nb, lo, hi):
                w_sb = wq16_sb if pi == 0 else wk16_sb
                for kc in range(lo, hi):
                    nc.tensor.matmul(
                        ps,
                        lhsT=w_sb[:, mc * MB + kc * 128 : mc * MB + kc * 128 + 128],
                        rhs=xt[:, kc * XBW + nb * QB : kc * XBW + (nb + 1) * QB],
                        start=(kc == 0),
                        stop=(kc == FKC - 1),
                    )

            def _qk_drain(ps, pi, mc, nb, drain_on_act):
                out_sb = qt if pi == 0 else kt
                if drain_on_act:
                    nc.scalar.activation(
                        out=out_sb[:, mc * S + nb * QB : mc * S + (nb + 1) * QB],
                        in_=ps,
                        func=mybir.ActivationFunctionType.Identity,
                        bias=bqk_sb[:, pi * MC + mc : pi * MC + mc + 1],
                        scale=1.0,
                    )
                else:
                    nc.vector.tensor_scalar_add(
                        out_sb[:, mc * S + nb * QB : mc * S + (nb + 1) * QB],
                        ps,
                        bqk_sb[:, pi * MC + mc : pi * MC + mc + 1],
                    )

            def finish_half(fq, drain_on_act=True):
                ps = half_pending.pop(fq)
                _qk_bf16_mms(ps, fq[1], fq[2], fq[3], FKC // 2, FKC)
                _qk_drain(ps, fq[1], fq[2], fq[3], drain_on_act)

            def qk_quantum(pi, mc, nb, drain_on_act=False, first_half_only=False):
                kind_bf16 = (pi == 0 and nb == 0) or (pi == 1 and nb <= 1)
                ps = pps.tile([128, QB], F32, tag="proj", name="psqk")
                if kind_bf16:
                    if first_half_only:
                        _qk_bf16_mms(ps, pi, mc, nb, 0, FKC // 2)
                        half_pending[("qk", pi, mc, nb)] = ps
                        return
                    _qk_bf16_mms(ps, pi, mc, nb, 0, FKC)
                else:
                    w4 = wq84 if pi == 0 else wk84
                    for j in range(FKC // 2):
                        nc.tensor.matmul(
                            ps,
                            lhsT=w4[:, mc, 2 * j : 2 * j + 2, :],
                            rhs=xt84[:, 2 * j : 2 * j + 2, nb * QB : (nb + 1) * QB],
                            start=(j == 0),
                            stop=(j == FKC // 2 - 1),
                            perf_mode=DR,
                        )
                _qk_drain(ps, pi, mc, nb, drain_on_act)

            def v_quantum(sc):
                ps = pps.tile([128, HD], F32, tag="proj", name="psv")
                if sc < N0:
                    for kc in range(FKC):
                        nc.tensor.matmul(
                            ps,
                            lhsT=xt[:, kc * XBW + sc * KC : kc * XBW + sc * KC + KC],
                            rhs=wv16_sb[:, kc * HD : (kc + 1) * HD],
                            start=(kc == 0),
                            stop=(kc == FKC - 1),
                        )
                else:
                    for j in range(FKC // 2):
                        nc.tensor.matmul(
                            ps,
                            lhsT=xt84[:, 2 * j : 2 * j + 2, sc * KC : sc * KC + KC],
                            rhs=wv84[:, 2 * j : 2 * j + 2, :],
                            start=(j == 0),
                            stop=(j == FKC // 2 - 1),
                            perf_mode=DR,
                        )
                nc.vector.tensor_add(
                    v4[:, sc, :, 0:D],
                    ps.rearrange("p (h e) -> p h e", e=D),
                    bv_sb.rearrange("p (h e) -> p h e", e=D),
                )
                if sc in vb_chunks:
                    nc.vector.tensor_add(
                        vb4[:, sc, :, 0:D],
                        ps.rearrange("p (h e) -> p h e", e=D),
                        bv_sb.rearrange("p (h e) -> p h e", e=D),
                    )

            emitted = set()

            def flush_halves(drain_on_act=True):
                # a pending half holds a pps ring buffer; its completing
                # matmuls+drain must precede any further pps allocation in
                # the in-order PE FIFO or a later allocation can deadlock
                for f in list(half_pending):
                    finish_half(f, drain_on_act)
                    emitted.add(f)

            def emit_quantum(fq, drain_on_act=False, first_half=False):
                if fq in emitted or fq in half_pending:
                    return
                flush_halves()
                kind_bf16 = fq[0] == "qk" and (
                    (fq[1] == 0 and fq[3] == 0) or (fq[1] == 1 and fq[3] <= 1)
                )
                if fq[0] == "v":
                    emitted.add(fq)
                    v_quantum(fq[1])
                    return
                if first_half and kind_bf16:
                    qk_quantum(fq[1], fq[2], fq[3], drain_on_act, first_half_only=True)
                    return
                emitted.add(fq)
                qk_quantum(fq[1], fq[2], fq[3], drain_on_act)

            # filler order: mc0 fp8 quanta, then the HEAVY bf16 quanta of
            # mc1/mc2 spread early (so the hp transitions never demand a
            # burst of 1.3us quanta at once), then v tail, then fp8 mc1/2.
            filler = []
            for nb in range(1, NQB):
                filler.append(("qk", 0, 0, nb))
            for nb in range(2, NQB):
                filler.append(("qk", 1, 0, nb))
            for sc in range(N0, N0 + 4):
                filler.append(("v", sc))
            for mc in (1, 2):
                filler.append(("qk", 0, mc, 0))
                filler.append(("qk", 1, mc, 0))
                filler.append(("qk", 1, mc, 1))
            for sc in range(N0 + 4, NKC):
                filler.append(("v", sc))
            for mc in (1, 2):
                for pi in range(2):
                    for nb in range(1 if pi == 0 else 2, NQB):
                        filler.append(("qk", pi, mc, nb))

            demand_q = []

            def require(fq):
                if fq in emitted:
                    return
                if fq in half_pending:
                    finish_half(fq, drain_on_act=False)  # demanded: DVE drain
                    emitted.add(fq)
                    return
                if fq in filler:
                    filler.remove(fq)
                if fq in demand_q:
                    demand_q.remove(fq)
                emit_quantum(fq)

            def require_soon(fq):
                if fq in emitted or fq in half_pending or fq in demand_q:
                    return
                if fq in filler:
                    filler.remove(fq)
                demand_q.append(fq)

            def warm_dummy():
                ps = pps.tile([128, QB], F32, tag="proj", name="warm")
                nc.tensor.matmul(
                    ps,
                    lhsT=warmsrc[:, 0:128],
                    rhs=warmsrc[:, 128 : 128 + QB],
                    start=True,
                    stop=True,
                )

            def leisure_pop():
                if half_pending:
                    flush_halves()
                elif demand_q:
                    emit_quantum(demand_q.pop(0))
                elif filler:
                    # NB: half-quantum smoothing (first_half=True) tested
                    # 3.3us WORSE than whole-quantum pops; keep whole.
                    emit_quantum(filler.pop(0), drain_on_act=True)
                else:
                    for _ in range(4):
                        warm_dummy()

            def scores_needs(hp, qb, g):
                needs = [("qk", 0, hp, qb)]
                nb_hi = (g[-1] * KC + KC - 1) // QB
                for nb in range(nb_hi + 1):
                    needs.append(("qk", 1, hp, nb))
                return needs

            def group_needs(hp, qb, g):
                return scores_needs(hp, qb, g) + [("v", kc) for kc in g]

            # 6 dependency-free dummies run while the input DMAs are in
            # flight, ramping the PE p-state so the prelude quanta execute
            # at ~325ns instead of cold-clock ~600ns.  (Tested alone: the
            # earlier +3.8us "pre-warm failure" was a bundled xt-on-Scalar
            # change, not the warm-up itself.)
            for _ in range(6):
                warm_dummy()
            emit_quantum(("qk", 0, 0, 0))
            emit_quantum(("qk", 1, 0, 0))
            for sc in range(N0):
                filler.insert(sc, ("v", sc))

            # --- attention -----------------------------------------------
            def emit_scores_group(hp, qb, g, gi):
                if qb == 0:
                    path = "bf16"
                elif (hp, qb, gi) in dve_groups:
                    path = "dve"
                else:
                    path = "fp8"
                q0s = [q0map[(kc, qb)] for kc in g]
                gq0 = min(q0s)
                # per-chunk exp ranges only on the fp8 path (its esb ring is
                # pre-zeroed; stale [gq0,q0c) is masked to 0, never NaN)
                exp_split = (max(q0s) - gq0) >= 224 and path == "fp8"
                eq0 = [q0 if (exp_split or q0 == gq0) else gq0 for q0 in q0s]

                for kc in g:
                    if (kc, qb) in bnd_index:
                        mask_jit(kc, qb)
                mcq = hp
                sps = {}
                esb = {}
                etag = {"bf16": "esb16b", "fp8": "esb8", "dve": "esb16"}[path]
                edt = FP8 if path == "fp8" else BF16
                for par in range(2):
                    sps[par] = scps.tile(
                        [128, ACT_GROUP * QB], F32, tag="sps", name=f"sps{par}"
                    )
                    esb[par] = expp.tile(
                        [128, ACT_GROUP * QB], edt, tag=etag, name=f"esb{par}"
                    )
                # par-outer: par0's scores finish 1 matmul earlier, so its
                # exp (the slot's pacing chain) starts sooner; each par's
                # masks follow its own exp so ctx(par0) is ready while
                # exp(par1) still runs.
                scale = 1.0 / float(np.sqrt(np.float32(D)))
                for par in range(2):
                    po = par * 64
                    for j, kc in enumerate(g):
                        nc.tensor.matmul(
                            sps[par][:, j * QB + eq0[j] : (j + 1) * QB],
                            lhsT=kt[
                                po : po + 64, mcq * S + kc * KC : mcq * S + kc * KC + KC
                            ],
                            rhs=qt[
                                po : po + 64,
                                mcq * S + qb * QB + eq0[j] : mcq * S + (qb + 1) * QB,
                            ],
                            start=True,
                            stop=True,
                        )
                for par in range(2):
                    if path == "dve":
                        nc.vector.tensor_scalar(
                            out=esb[par][:, 0 : len(g) * QB].bitcast(I16),
                            in0=sps[par][:, 0 : len(g) * QB],
                            scalar1=SCH_A,
                            scalar2=SCH_B,
                            op0=mybir.AluOpType.mult,
                            op1=mybir.AluOpType.add,
                        )
                    elif exp_split or len(g) == 1:
                        for j in range(len(g)):
                            nc.scalar.activation(
                                out=esb[par][:, j * QB + eq0[j] : (j + 1) * QB],
                                in_=sps[par][:, j * QB + eq0[j] : (j + 1) * QB],
                                func=mybir.ActivationFunctionType.Exp,
                                scale=scale,
                            )
                    else:
                        src = sps[par].rearrange("p (j q) -> p j q", j=2)[:, :, gq0:]
                        dst = esb[par].rearrange("p (j q) -> p j q", j=2)[:, :, gq0:]
                        nc.scalar.activation(
                            out=dst,
                            in_=src,
                            func=mybir.ActivationFunctionType.Exp,
                            scale=scale,
                        )
                    for j, kc in enumerate(g):
                        if (kc, qb) not in bnd_index:
                            continue
                        q1 = q1map[(kc, qb)]
                        if qb == 0:
                            bi, srcm = mi16[(kc, qb)], mskb
                        else:
                            bi, srcm = mi8[(kc, qb)], msk
                        nc.vector.tensor_mul(
                            esb[par][:, j * QB + gq0 : j * QB + q1],
                            esb[par][:, j * QB + gq0 : j * QB + q1],
                            srcm[:, bi * QB + gq0 : bi * QB + q1],
                        )
                return esb, gq0, path

            def emit_ctx_group(hp, qb, g, gq0, path, esb, cps, unit, n_units):
                if path == "fp8":
                    for par in range(2):
                        h = 2 * hp + par
                        if len(g) == 2:
                            nc.tensor.matmul(
                                cps[par][:, gq0:],
                                lhsT=v4[:, g[0] : g[0] + 2, h, 0 : D + 1],
                                rhs=esb[par].rearrange("p (j q) -> p j q", j=2)[
                                    :, :, gq0:
                                ],
                                start=(unit == 0),
                                stop=(unit == n_units - 1),
                                perf_mode=DR,
                            )
                        else:
                            nc.tensor.matmul(
                                cps[par][:, gq0:],
                                lhsT=v4[:, g[0], h, 0 : D + 1],
                                rhs=esb[par][:, gq0:QB],
                                start=(unit == 0),
                                stop=(unit == n_units - 1),
                            )
                    return unit + 1
                for j, kc in enumerate(g):
                    for par in range(2):
                        h = 2 * hp + par
                        nc.tensor.matmul(
                            cps[par][:, gq0:],
                            lhsT=vb16[
                                :, kc * VW16 + h * (D + 1) : kc * VW16 + (h + 1) * (D + 1)
                            ],
                            rhs=esb[par][:, j * QB + gq0 : (j + 1) * QB],
                            start=(unit + j == 0),
                            stop=(unit + j == n_units - 1),
                        )
                return unit + len(g)

            def n_units_for(hp, qb):
                n = 0
                for gi, g in enumerate(groups[(hp, qb)]):
                    if qb == 0 or (hp, qb, gi) in dve_groups:
                        n += len(g)
                    else:
                        n += 1
                return n

            all_slots = []
            for hp in range(HPC // 2):
                for qb in range(NQB):
                    for gi, g in enumerate(groups[(hp, qb)]):
                        all_slots.append((hp, qb, g))
            si = 0
            for fut in all_slots[0:2]:
                for need in group_needs(*fut):
                    require(need)

            def emit_drain(hp_d, qb_d, cps_d):
                for par in range(2):
                    po = par * 64
                    # NB: the custom-DVE reciprocal misreads on HW when the
                    # input base partition differs from the output's, so the
                    # l row is staged to partition 0 first - on the ACT
                    # engine, which has slack here, not the busy DVE.
                    lt = lpool.tile([1, QB], F32, tag="lt", name="lt")
                    lr = lpool.tile([1, QB], F32, tag="lr", name="lr")
                    bc = lpool.tile([64, QB], F32, tag="bc", name="bc")
                    nc.vector.tensor_copy(lt, cps_d[par][64:65, :])
                    nc.vector.reciprocal_approx_fast(out=lr, in_=lt)
                    nc.gpsimd.partition_broadcast(bc, lr)
                    nc.vector.tensor_mul(
                        ctxt[
                            po : po + 64,
                            hp_d * S + qb_d * QB : hp_d * S + (qb_d + 1) * QB,
                        ],
                        cps_d[par][0:64, :],
                        bc,
                    )
                nc.sync.dma_start(
                    out=out_d[
                        hp_d * 128 : (hp_d + 1) * 128, qb_d * QB : (qb_d + 1) * QB
                    ],
                    in_=ctxt[:, hp_d * S + qb_d * QB : hp_d * S + (qb_d + 1) * QB],
                )

            pending_drain = None
            for hp in range(HPC // 2):
                for qb in range(NQB):
                    gs = groups[(hp, qb)]
                    n_units = n_units_for(hp, qb)
                    cps = {}
                    for par in range(2):
                        cps[par] = ctxps.tile([65, QB], F32, tag="cps", name=f"cps{par}")
                    unit = 0
                    prev = None
                    for gi, g in enumerate(gs):
                        for need in scores_needs(hp, qb, g):
                            require(need)
                        esb, gq0, path = emit_scores_group(hp, qb, g, gi)
                        if gi == 0 and pending_drain is not None:
                            emit_drain(*pending_drain)
                            pending_drain = None
                        for fut in all_slots[si + 1 : si + 7]:
                            for need in group_needs(*fut):
                                require_soon(need)
                        leisure_pop()
                        si += 1
                        if prev is not None:
                            for kc in prev[0]:
                                require(("v", kc))
                            unit = emit_ctx_group(hp, qb, *prev, cps, unit, n_units)
                        prev = (g, gq0, path, esb)
                    leisure_pop()
                    for kc in prev[0]:
                        require(("v", kc))
                    unit = emit_ctx_group(hp, qb, *prev, cps, unit, n_units)
                    pending_drain = (hp, qb, cps)
            flush_halves()
            emit_drain(*pending_drain)

    nc.finalize()
    return nc


def _core_inputs(x, segment_ids, Wq, bq, Wk, bk, Wv, bv, cs, core):
    b, h0 = core // 2, HPC * (core % 2)
    cols = slice(h0 * D, (h0 + HPC) * D)
    np_fp8 = mybir.dt.np(FP8)
    xT = np.ascontiguousarray(x[b].T)  # [768, 2048]
    xT_s = xT.reshape(FKC, 128, S).transpose(1, 0, 2).reshape(128, FKC * S)
    xT_bf = xT_s.reshape(128, FKC, S)[:, :, 0 : XBQ * QB].reshape(
        128, FKC * XBQ * QB
    )

    def wprep_mc(Wm):
        ws = Wm[:, cols]
        arr = ws.reshape(FKC, 128, MC, 128).transpose(1, 2, 0, 3)
        return np.ascontiguousarray(arr.reshape(128, MC * MB))

    def wprep_kc(Wm):
        ws = Wm[:, cols]
        arr = ws.reshape(FKC, 128, HD).transpose(1, 0, 2)
        return np.ascontiguousarray(arr.reshape(128, FKC * HD))

    bq_s = np.ascontiguousarray(bq[cols].reshape(MC, 128).T)
    bk_s = np.ascontiguousarray(bk[cols].reshape(MC, 128).T)
    bqk = np.concatenate([bq_s, bk_s], axis=1)
    bvb = np.ascontiguousarray(np.broadcast_to(bv[cols], (128, HD)))
    csf = cs[b].astype(np.float32)
    cs_bcast = np.ascontiguousarray(np.broadcast_to(csf, (128, S)))
    cs_part = np.ascontiguousarray(csf.reshape(NKC, KC).T)
    wq_mc = wprep_mc(Wq)
    wk_mc = wprep_mc(Wk)
    wv_kc = wprep_kc(Wv)
    return {
        "xT": np.ascontiguousarray(xT_bf).astype(ml_dtypes.bfloat16),
        "xT8": xT_s.astype(np_fp8),
        "wq16": wq_mc.astype(ml_dtypes.bfloat16),
        "wk16": wk_mc.astype(ml_dtypes.bfloat16),
        "wq8": wq_mc.astype(np_fp8),
        "wk8": wk_mc.astype(np_fp8),
        "wv16": wv_kc.astype(ml_dtypes.bfloat16),
        "wv8": wv_kc.astype(np_fp8),
        "bqk": np.ascontiguousarray(bqk),
        "bvb": bvb,
        "cs_bcast": cs_bcast,
        "cs_part": cs_part,
    }


def kernel(x, segment_ids, Wq, bq, Wk, bk, Wv, bv):
    global LAST_RESULTS
    x = np.asarray(x, np.float32)
    segment_ids = np.asarray(segment_ids)
    Wq, bq = np.asarray(Wq, np.float32), np.asarray(bq, np.float32)
    Wk, bk = np.asarray(Wk, np.float32), np.asarray(bk, np.float32)
    Wv, bv = np.asarray(Wv, np.float32), np.asarray(bv, np.float32)

    cs, vis_lists, bnd_index, qmaps = _classify(segment_ids)
    nc = _build_program(vis_lists, bnd_index, qmaps)
    in_maps = [
        _core_inputs(x, segment_ids, Wq, bq, Wk, bk, Wv, bv, cs, c)
        for c in range(NCORES)
    ]
    if TRACE:
        _ensure_ntff_hook()
    res = run_bass_kernel_spmd(nc, in_maps, list(range(NCORES)), trace=TRACE)
    LAST_RESULTS = res

    out = np.empty((B, S, W), np.float32)
    for c in range(NCORES):
        b, h0 = c // 2, HPC * (c % 2)
        out[b, :, h0 * D : (h0 + HPC) * D] = res.results[c]["ctxT"].T
    return out
